# revision 22
# baseline (speedup 1.0000x reference)
"""Differential cross-attention Bass kernel for 8 Trainium2 NeuronCores.

Sharding: heads are split across cores (2 of 16 heads per core). Each core
computes Q/K/V projections for its head slice, both N x N differential score
maps for its (batch, head) units, softmax (no max-subtraction; scores are
O(1) so exp is safe), attn = a1 - lam*a2, GroupNorm per (b, h), and a partial
output projection against its 128-column slice of Wo. The host sums the 8
partial outputs and adds the output bias.

Performance-critical design notes (v2):
- All big matmuls are fp16 with FULL k=128 contraction (HAM clock governor
  ignores <128-row matmuls). K stays dim-major as lhsT; Q is stored as four
  zero-masked variants so every score matmul is a standard 128x128 matmul.
- The exp stream (67M PSUM f32 elements/core) is THE co-bottleneck with the
  PE: it is split across BOTH ScalarE (exact spline exp, map1 columns plus a
  tunable slice of map2) and the DVE (map2 columns via a one-instruction
  Schraudolph: i16 = round(s*A + B), bits reinterpreted as fp16; the e^K
  range-shift and the sawtooth's mean cancel in the softmax division, and
  the remaining +-2.9% sawtooth error only enters scaled by lambda~0.36).
- All staging after PSUM is fp16: o12 is evacuated by ScalarE as fp16, the
  per-block transposes and tail transposes run at 1 cyc/row instead of 2,
  and the combine/affine DVE ops hit the 2x_1P packed mode. Partial outputs
  DMA to DRAM as fp16 (halves the output traffic; host sums in f64).
- Per-block combine: ONE batched reciprocal over all 8 (sub, map)
  denominators (strided AP into the single fp16 transpose tile), then
  ts+stt per sub. GroupNorm stats stay off PE/ACT (GpSimd all-reduce, DVE
  bit-trick rsqrt), computed per-head right after the head's blocks.
- Unit tails are split in half (h0 transposes+affine | h1 + Wo + output) and
  popped at two points of the NEXT unit; the LAST unit pops its h0 half
  during its own h1 blocks, shrinking the end-of-kernel half-clock window.
- Projection phase: x/wq DMAs are issued first, the q-variant zero-fills run
  on the idle GpSimd engine, and PSUM evacuation is split between ScalarE
  (Identity with per-partition bias AP) and the DVE so neither engine gates
  the x-chunk pipeline.
"""

import os
import sys
from contextlib import ExitStack

import numpy as np

for _p in ("/opt/trn_rl_repo", "/opt/pypackages"):
    if os.path.isdir(_p) and _p not in sys.path:
        sys.path.append(_p)

import concourse.bass as bass
import concourse.bass_isa as bass_isa
import concourse.tile as tile
from concourse import bacc, mybir
from concourse.bass_utils import run_bass_kernel_spmd

# ---- problem constants (hardcoded per contest contract) ----
B, N, DIM, H, HEAD, HALF = 2, 2048, 1024, 16, 64, 32
SCALE = HALF ** -0.5
LAMBDA_INIT = 0.8 - 0.6 * float(np.exp(-0.3 * (2 - 1)))
EPS = 1e-5
NCORES = 8
HPC = H // NCORES          # 2 heads per core
DC = HPC * HEAD            # 128 feature dims per core
NT = B * N                 # 4096 tokens
F32 = mybir.dt.float32
F16 = mybir.dt.float16
I16 = mybir.dt.int16

_DTMAP = {
    "float16": mybir.dt.float16,
    "bfloat16": mybir.dt.bfloat16,
    "float32r": mybir.dt.float32r,
    "float32": mybir.dt.float32,
}
MM_DT = _DTMAP[os.environ.get("BASS_MM_DT", "float16")]

# exp split: ACT handles map1 (cols 0:512 of each score tile) exactly; the
# DVE handles map2 (cols 512:1024) via Schraudolph. Separate p1/p2 tiles so
# the two engines run concurrently (a shared tile serializes the writers).
# Schraudolph fp16 constants: bits = v*SA + SB, v = s*SCALE (folded into SA)
SHIFT_K = 1.0              # extra e^K factor, cancels per-column in softmax
# (K=1: max fp16-staged denominator ~22k on the actual data, 3x margin;
#  negative scores stay out of fp16-subnormal territory down to v=-9.9)
SCHR_C = float(os.environ.get("BASS_SCHR_C", "-0.0434"))
SA = float(1024.0 / np.log(2.0)) * SCALE
SB = float((15.0 + SCHR_C) * 1024.0 + SHIFT_K * 1024.0 / np.log(2.0))

LAST_EXEC_NS = None
_PROG_CACHE = {}


def _build_kernel(nc):
    AF = mybir.ActivationFunctionType
    ALU = mybir.AluOpType
    AX = mybir.AxisListType

    x1T = nc.dram_tensor("x1T", (DIM, NT), MM_DT, kind="ExternalInput").ap()
    x2T = nc.dram_tensor("x2T", (DIM, NT), MM_DT, kind="ExternalInput").ap()
    wqT = nc.dram_tensor("wqT", (DIM, DC), MM_DT, kind="ExternalInput").ap()
    wkT = nc.dram_tensor("wkT", (DIM, DC), MM_DT, kind="ExternalInput").ap()
    wvT = nc.dram_tensor("wvT", (DIM, DC), MM_DT, kind="ExternalInput").ap()
    woT = nc.dram_tensor("woT", (DC, DIM), MM_DT, kind="ExternalInput").ap()
    bqv = nc.dram_tensor("bqv", (DC, 1), F32, kind="ExternalInput").ap()
    bkv = nc.dram_tensor("bkv", (DC, 1), F32, kind="ExternalInput").ap()
    bvv = nc.dram_tensor("bvv", (DC, 1), F32, kind="ExternalInput").ap()
    lamn = nc.dram_tensor("lamn", (128, HPC), F32, kind="ExternalInput").ap()
    vones = nc.dram_tensor("vones", (128, 32, 65), MM_DT, kind="ExternalInput").ap()
    identr = nc.dram_tensor("identr", (128, 128), MM_DT, kind="ExternalInput").ap()
    gwv = nc.dram_tensor("gwv", (DC, 1), F32, kind="ExternalInput").ap()
    gbv = nc.dram_tensor("gbv", (DC, 1), F32, kind="ExternalInput").ap()
    gnc = nc.dram_tensor(
        "gnc", (128, 2), mybir.dt.uint32, kind="ExternalInput"
    ).ap()
    out1p = nc.dram_tensor("out1p", (NT, DIM), F16, kind="ExternalOutput").ap()
    out2p = nc.dram_tensor("out2p", (NT, DIM), F16, kind="ExternalOutput").ap()

    with tile.TileContext(nc) as tc, ExitStack() as top:
        consts = top.enter_context(tc.tile_pool(name="consts", bufs=1))
        qkpool = top.enter_context(tc.tile_pool(name="qkpool", bufs=1))
        vpool = top.enter_context(tc.tile_pool(name="vpool", bufs=1))

        # ---- constants; wq + the first x chunk DMA first so the first
        # projection matmul starts as early as possible
        wq_t = consts.tile([128, 8, DC], MM_DT, tag="wq")
        wk_t = consts.tile([128, 8, DC], MM_DT, tag="wk")
        wv_t = consts.tile([128, 8, DC], MM_DT, tag="wv")
        nc.sync.dma_start(wq_t, wqT.rearrange("(kc p) d -> p kc d", p=128))
        x1Tr = x1T.rearrange("(kc p) t -> p kc t", p=128)
        x2Tr = x2T.rearrange("(kc p) t -> p kc t", p=128)
        xpre = consts.tile([128, 8, 1024], MM_DT, tag="xpre")
        nc.sync.dma_start(xpre, x1Tr[:, :, 0:1024])
        nc.sync.dma_start(wk_t, wkT.rearrange("(kc p) d -> p kc d", p=128))
        nc.sync.dma_start(wv_t, wvT.rearrange("(kc p) d -> p kc d", p=128))
        bq_t = consts.tile([DC, 1], F32, tag="bq")
        bk_t = consts.tile([DC, 1], F32, tag="bk")
        bv_t = consts.tile([DC, 1], F32, tag="bv")
        nc.sync.dma_start(bq_t, bqv)
        nc.sync.dma_start(bk_t, bkv)
        nc.sync.dma_start(bv_t, bvv)
        identr_t = consts.tile([128, 128], MM_DT, tag="identr")
        nc.sync.dma_start(identr_t, identr)
        wo_t = consts.tile([DC, DIM], MM_DT, tag="wo")
        nc.sync.dma_start(wo_t, woT)
        lam_t = consts.tile([128, HPC], F32, tag="lam")
        nc.sync.dma_start(lam_t, lamn)
        gw_t = consts.tile([DC, 1], F32, tag="gw")
        gb_t = consts.tile([DC, 1], F32, tag="gb")
        nc.sync.dma_start(gw_t, gwv)
        nc.sync.dma_start(gb_t, gbv)
        # uint32 constants for the DVE-only rsqrt: [0x5F3759DF magic, 1]
        gnc_t = consts.tile([128, 2], mybir.dt.uint32, tag="gnc")
        nc.sync.dma_start(gnc_t, gnc)

        # K dim-major; Q as 4 zero-masked variants per tensor (head x half)
        k1_t = qkpool.tile([128, NT], MM_DT, tag="k1")
        k2_t = qkpool.tile([128, NT], MM_DT, tag="k2")
        q1_v = [qkpool.tile([128, 2, NT], MM_DT, name=f"q1v{i}", tag=f"q1v{i}")
                for i in range(HPC)]
        q2_v = [qkpool.tile([128, 2, NT], MM_DT, name=f"q2v{i}", tag=f"q2v{i}")
                for i in range(HPC)]
        # variant zero-fill on the otherwise-idle GpSimd engine (keeps the
        # 4x ~7us memsets off the DVE's critical path during warmup)
        for v in q1_v + q2_v:
            nc.gpsimd.memset(v, 0.0)
        # V token-major: (tok 128, chunk 32, [64 h0 | 1 | 64 h1 | 1 | 63 pad])
        v1_t = vpool.tile([128, 32, 193], MM_DT, tag="v1")
        v2_t = vpool.tile([128, 32, 193], MM_DT, tag="v2")
        # ================= phase P: projections =================
        with ExitStack() as ph:
            xin = ph.enter_context(tc.tile_pool(name="xin", bufs=3))
            pqk = ph.enter_context(tc.tile_pool(name="pqk", bufs=3, space="PSUM"))
            pv = ph.enter_context(tc.tile_pool(name="pv", bufs=2, space="PSUM"))
            for xTr, qv, kd, vd in (
                (x1Tr, q1_v, k1_t, v1_t), (x2Tr, q2_v, k2_t, v2_t)
            ):
                for tcn in range(4):
                    ts0 = tcn * 1024
                    if xTr is x1Tr and tcn == 0:
                        xt = xpre  # prefetched before the consts DMAs
                    else:
                        xt = xin.tile([128, 8, 1024], MM_DT, tag="x")
                        nc.sync.dma_start(xt, xTr[:, :, ts0 : ts0 + 1024])
                    vstage = xin.tile([128, 1024], MM_DT, tag="vs")
                    for wt, bt, dst in (
                        (wq_t, bq_t, None), (wk_t, bk_t, kd), (wv_t, bv_t, vstage)
                    ):
                        ps = pqk.tile([128, 1024], F32, tag="qk")
                        for kc in range(8):
                            for jh in range(2):
                                nc.tensor.matmul(
                                    ps[:, jh * 512 : (jh + 1) * 512],
                                    lhsT=wt[:, kc, :],
                                    rhs=xt[:, kc, jh * 512 : (jh + 1) * 512],
                                    start=(kc == 0),
                                    stop=(kc == 7),
                                )
                        if dst is None:
                            # Q: scatter rows into the zero-masked head pairs
                            # (split ACT/DVE so neither engine gates the loop)
                            for h in range(HPC):
                                for j in range(2):
                                    hs = slice(h * 64 + j * 32, h * 64 + j * 32 + 32)
                                    dstap = qv[h][hs, j, ts0 : ts0 + 1024]
                                    if j == 0:
                                        nc.scalar.activation(
                                            dstap, ps[hs, :], AF.Identity,
                                            bias=bt[hs, :],
                                        )
                                    else:
                                        nc.vector.tensor_scalar(
                                            dstap, ps[hs, :], bt[hs, :], None,
                                            ALU.add,
                                        )
                        else:
                            out_ap = (
                                dst if dst is vstage else dst[:, ts0 : ts0 + 1024]
                            )
                            nc.scalar.activation(
                                out_ap, ps, AF.Identity, bias=bt
                            )
                    # transpose V chunk to token-major and scatter into V tile
                    for sc in range(8):
                        tp = pv.tile([128, 128], MM_DT, tag="v")
                        nc.tensor.transpose(
                            tp, vstage[:, sc * 128 : (sc + 1) * 128], identr_t
                        )
                        sg = tcn * 8 + sc
                        nc.vector.tensor_copy(vd[:, sg, 0:64], tp[:, 0:64])
                        nc.vector.tensor_copy(vd[:, sg, 65:129], tp[:, 64:128])
            for vd in (v1_t, v2_t):
                nc.sync.dma_start(vd[:, :, 64:65], vones[:, :, 0:1])
                nc.sync.dma_start(vd[:, :, 129:130], vones[:, :, 1:2])
                nc.sync.dma_start(vd[:, :, 130:193], vones[:, :, 2:65])
        # ================= phase A: attention =================
        with ExitStack() as ph:
            pp = ph.enter_context(tc.tile_pool(name="pp", bufs=6))
            osb = ph.enter_context(tc.tile_pool(name="osb", bufs=2))
            ocomb = ph.enter_context(tc.tile_pool(name="ocomb", bufs=2))
            otp = ph.enter_context(tc.tile_pool(name="otp", bufs=2))
            wos = ph.enter_context(tc.tile_pool(name="wos", bufs=4))
            small = ph.enter_context(tc.tile_pool(name="small", bufs=8))
            ps_s = ph.enter_context(tc.tile_pool(name="ps_s", bufs=2, space="PSUM"))
            ps_o = ph.enter_context(tc.tile_pool(name="ps_o", bufs=1, space="PSUM"))
            ps_t = ph.enter_context(tc.tile_pool(name="ps_t", bufs=2, space="PSUM"))

            # pending tails: (T1, T2, T3) closures per unit. T1 handles subs
            # 0..7 (transpose+affine), T2 subs 8..15, T3 the Wo projection +
            # output DMA. Non-last units pop T1 at (h0,qc1) and T2+T3 at
            # (h0,qc3) of the NEXT unit. The LAST unit runs its own T1 at
            # (h1,qc2) with the h0 affine rows only (h1 stats aren't ready),
            # and finishes the rest at the very end. All transposes are full
            # 128-partition ops (64-row ops downclock the PE via HAM).
            pending = []

            def make_tails(ot, ocu, sdv, tdv, b, outp):
                def subs_pass(s0, s1, rows):
                    for sub in range(s0, s1):
                        tb = ps_t.tile([128, 128], MM_DT, tag="t")
                        nc.tensor.transpose(tb, ocu[:, sub, :], identr_t)
                        nc.vector.tensor_scalar(
                            ot[rows, sub * 128 : (sub + 1) * 128],
                            tb[rows, :], sdv[rows, :], tdv[rows, :],
                            ALU.mult, ALU.add,
                        )

                def T1():
                    subs_pass(0, 8, slice(0, 128))

                def T1_last():
                    subs_pass(0, 8, slice(0, 64))

                def T1_last_fixup():
                    # affine for h1 rows of subs 0..7 (tb tiles are gone, so
                    # re-transpose is avoided by affining from a fresh pass)
                    for sub in range(8):
                        tb = ps_t.tile([128, 128], MM_DT, tag="t")
                        nc.tensor.transpose(tb, ocu[:, sub, :], identr_t)
                        nc.vector.tensor_scalar(
                            ot[64:128, sub * 128 : (sub + 1) * 128],
                            tb[64:128, :], sdv[64:128, :], tdv[64:128, :],
                            ALU.mult, ALU.add,
                        )

                def T2():
                    subs_pass(8, 16, slice(0, 128))

                def T3_piece(p):
                    def run():
                        for tl in range(4 * p, 4 * p + 4):
                            for nc2 in range(2):
                                wp = ps_t.tile([128, 512], F32, tag="t")
                                nc.tensor.matmul(
                                    wp,
                                    lhsT=ot[:, tl * 128 : (tl + 1) * 128],
                                    rhs=wo_t[:, nc2 * 512 : (nc2 + 1) * 512],
                                    start=True,
                                    stop=True,
                                )
                                ws = wos.tile([128, 512], F16, tag="ws")
                                # alternate evacuation engine to balance load
                                if (tl * 2 + nc2) % 2 == 0:
                                    nc.scalar.activation(ws, wp, AF.Copy)
                                else:
                                    nc.vector.tensor_copy(ws, wp)
                                nc.sync.dma_start(
                                    outp[
                                        b * N + tl * 128 : b * N + (tl + 1) * 128,
                                        nc2 * 512 : (nc2 + 1) * 512,
                                    ],
                                    ws,
                                )
                    return run
                return (T1, T1_last, T1_last_fixup, T2,
                        [T3_piece(p) for p in range(4)])

            units = [(b, a) for b in range(B) for a in range(2)]
            for ui, (b, a) in enumerate(units):
                last = ui == len(units) - 1
                qv = q1_v if a == 0 else q2_v
                kt = k2_t if a == 0 else k1_t
                vt = v2_t if a == 0 else v1_t
                outp = out1p if a == 0 else out2p
                ot = otp.tile([128, N], MM_DT, tag="ot")
                # combined, normalized attn output for BOTH heads:
                # (q 128, sub 16, [h0 64 | h1 64]) in fp16
                ocu = ocomb.tile([128, 16, 128], F16, tag="oc")
                sdv = small.tile([128, 1], F32, tag="sdv")
                tdv = small.tile([128, 1], F32, tag="tdv")
                my_tails = make_tails(ot, ocu, sdv, tdv, b, outp)
                for h in range(HPC):
                    hb = h * 64
                    for qc in range(4):
                        q0 = b * N + qc * 512
                        o12 = ps_o.tile([128, 1024], F32, tag="o")
                        va0 = h * 65  # [0:65]=h0 dims+ones, [65:130]=h1

                        # software-pipelined: S(kc) scores -> exp(kc) ->
                        # @V(kc-1), so PE never stalls on the exp engines.
                        # p1 (map1, exact ACT exp) and p2 (map2, DVE
                        # Schraudolph) are separate tiles so the two exp
                        # engines run concurrently.
                        p1tiles = [None] * 16
                        p2tiles = [None] * 16

                        def emit_scores(kc):
                            k0 = b * N + kc * 128
                            s = ps_s.tile([128, 1024], F32, tag="s")
                            # both halves against the same K slice;
                            # each matmul stays within one PSUM bank.
                            for j in range(2):
                                nc.tensor.matmul(
                                    s[:, j * 512 : (j + 1) * 512],
                                    lhsT=kt[:, k0 : k0 + 128],
                                    rhs=qv[h][:, j, q0 : q0 + 512],
                                    start=True,
                                    stop=True,
                                )
                            p1 = pp.tile([128, 512], MM_DT, tag="p1")
                            p2 = pp.tile([128, 512], MM_DT, tag="p2")
                            nc.scalar.activation(
                                p1, s[:, 0:512], AF.Exp, scale=SCALE,
                            )
                            nc.vector.tensor_scalar(
                                p2.bitcast(I16),
                                s[:, 512:1024],
                                SA, SB, ALU.mult, ALU.add,
                            )
                            p1tiles[kc] = p1
                            p2tiles[kc] = p2

                        def emit_av(kc):
                            vs = b * 16 + kc
                            for j, pt in ((0, p1tiles[kc]), (1, p2tiles[kc])):
                                nc.tensor.matmul(
                                    o12[:, j * 512 : (j + 1) * 512],
                                    lhsT=vt[:, vs, va0 : va0 + 128],
                                    rhs=pt,
                                    start=(kc == 0),
                                    stop=(kc == 15),
                                    skip_group_check=True,
                                )

                        STAG = 4
                        for kc in range(STAG):
                            emit_scores(kc)
                        for kc in range(STAG, 16):
                            emit_scores(kc)
                            emit_av(kc - STAG)
                        for kc in range(16 - STAG, 16):
                            emit_av(kc)
                        # interleave deferred tail pieces with the stream:
                        # T1 / T2 at (h0,qc1) / (h0,qc3); Wo in 4 pieces
                        # across the h1 blocks (a monolithic Wo burst paces
                        # the PE at PSUM-evacuation speed and lets HAM
                        # downclock).
                        if pending:
                            pt = pending[0]
                            if h == 0 and qc == 1:
                                pt[0]()              # prev unit T1
                            if h == 0 and qc == 3:
                                pt[3]()              # prev unit T2
                            if h == 1:
                                pt[4][qc]()          # prev unit T3 piece
                                if qc == 3:
                                    pending.pop(0)
                        if last and h == 1 and qc == 2:
                            my_tails[1]()            # own T1 (h0 rows only)
                        # evacuate o12 PSUM -> fp16 staging on ACT (1 op)
                        os_t = osb.tile([65, 1024], F16, tag="os")
                        nc.scalar.activation(os_t, o12[0:65, :], AF.Copy)
                        # transpose to (q, [sub, 65]) -- single fp16 PSUM
                        # bank, 68-col stride keeps 8B alignment
                        t12 = ps_t.tile([128, 8, 68], F16, tag="t")
                        for i in range(8):
                            nc.tensor.transpose(
                                t12[:, i, 0:65],
                                os_t[:, i * 128 : (i + 1) * 128],
                                identr_t[0:65, 0:65],
                            )
                        # batched combine: one reciprocal for all 8 denoms.
                        # The fp16->f32 copy is tiny; feeding fp16 into
                        # reciprocal directly makes walrus use a low-precision
                        # fp16 divide path (costs ~2x final accuracy).
                        den = small.tile([128, 8], F32, tag="den")
                        nc.vector.tensor_copy(den, t12[:, :, 64:65])
                        rec = small.tile([128, 8], F32, tag="rec")
                        nc.vector.reciprocal(rec, den)
                        s2p = small.tile([128, 4], F32, tag="s2p")
                        nc.vector.tensor_scalar(
                            s2p, rec[:, 4:8], lam_t[:, h : h + 1], None,
                            ALU.mult,
                        )
                        for i in range(4):
                            sidx = qc * 4 + i
                            tmp = small.tile([128, 64], F16, tag="tmp")
                            nc.vector.tensor_scalar(
                                tmp, t12[:, i, 0:64], rec[:, i : i + 1], None,
                                ALU.mult,
                            )
                            # ocu = (O2 * s2p) + tmp   (s2p = -lam / sum2)
                            nc.vector.scalar_tensor_tensor(
                                ocu[:, sidx, hb : hb + 64],
                                t12[:, 4 + i, 0:64],
                                s2p[:, i : i + 1],
                                tmp,
                                ALU.mult,
                                ALU.add,
                            )
                        if last and h == 1 and qc == 3:
                            # keep the PE busy while the final combine/GN
                            # chain drains, so HAM doesn't halve the clock
                            # for the whole end-of-kernel tail
                            for dk in range(24):
                                sd = ps_s.tile([128, 1024], F32, tag="s")
                                nc.tensor.matmul(
                                    sd[:, 0:512],
                                    lhsT=kt[:, b * N : b * N + 128],
                                    rhs=qv[h][:, 0, q0 : q0 + 512],
                                    start=True,
                                    stop=True,
                                )
                    # ---- GroupNorm stats for head h: free-dim sums ride
                    # along ACT activations via accum_out (a DVE reduce on
                    # fp16 input inserts big CAST instructions), then a
                    # GpSimd partition all-reduce + DVE bit-trick rsqrt.
                    st = small.tile([128, 2], F32, tag="st")
                    sq = ocomb.tile([128, 16, 64], F16, tag="sq")
                    nc.scalar.activation(
                        sq, ocu[:, :, hb : hb + 64], AF.Identity,
                        accum_out=st[:, 0:1],
                    )
                    nc.scalar.activation(
                        sq, ocu[:, :, hb : hb + 64], AF.Square,
                        accum_out=st[:, 1:2],
                    )
                    red = small.tile([128, 2], F32, tag="red")
                    nc.gpsimd.partition_all_reduce(
                        red, st, 128, bass_isa.ReduceOp.add
                    )
                    mr = small.tile([128, 2], F32, tag="mr")
                    nc.vector.tensor_scalar(
                        mr, red, 1.0 / (N * HEAD), None, ALU.mult
                    )
                    m2 = small.tile([128, 1], F32, tag="m2")
                    nc.vector.tensor_tensor(
                        m2, mr[:, 0:1], mr[:, 0:1], ALU.mult
                    )
                    var = small.tile([128, 1], F32, tag="var")
                    nc.vector.tensor_sub(var, mr[:, 1:2], m2)
                    # rstd via DVE-only bit-trick rsqrt (seed + 2 Newton)
                    hs = slice(h * 64, h * 64 + 64)
                    veps = small.tile([128, 1], F32, tag="veps")
                    nc.vector.tensor_scalar(veps, var, EPS, None, ALU.add)
                    vsh = small.tile([128, 1], mybir.dt.uint32, tag="vsh")
                    nc.vector.tensor_scalar(
                        vsh, veps.bitcast(mybir.dt.uint32),
                        gnc_t[:, 1:2], None, ALU.logical_shift_right,
                    )
                    y0i = small.tile([128, 1], mybir.dt.uint32, tag="y0i")
                    nc.vector.tensor_tensor(
                        y0i, gnc_t[:, 0:1], vsh, ALU.subtract
                    )
                    vh = small.tile([128, 1], F32, tag="vh")
                    nc.vector.tensor_scalar(vh, veps, 0.5, None, ALU.mult)
                    y = y0i.bitcast(F32)
                    for _ in range(2):  # Newton: y*(1.5-vh*y^2)
                        t1n = small.tile([128, 1], F32, tag="nt1")
                        nc.vector.tensor_tensor(t1n, y, y, ALU.mult)
                        t2n = small.tile([128, 1], F32, tag="nt2")
                        nc.vector.tensor_tensor(t2n, vh, t1n, ALU.mult)
                        t3n = small.tile([128, 1], F32, tag="nt3")
                        nc.vector.tensor_scalar(
                            t3n, t2n, 1.5, -1.0, ALU.subtract, ALU.mult
                        )
                        yn = small.tile([128, 1], F32, tag="yn")
                        nc.vector.tensor_tensor(yn, y, t3n, ALU.mult)
                        y = yn
                    tmp1 = small.tile([128, 1], F32, tag="tmp1")
                    nc.vector.tensor_tensor(
                        sdv[hs, :], gw_t[hs, :], y[hs, :], ALU.mult
                    )
                    nc.vector.tensor_tensor(
                        tmp1[hs, :], mr[hs, 0:1], sdv[hs, :], ALU.mult
                    )
                    nc.vector.tensor_sub(tdv[hs, :], gb_t[hs, :], tmp1[hs, :])
                if not last:
                    pending.append(my_tails)
            # final unit: T1 (h0 rows) already ran inline at (h1,qc2);
            # finish the h1 rows of subs 0..7, then subs 8..15 and Wo.
            my_tails[2]()
            my_tails[3]()
            for piece in my_tails[4]:
                piece()
    return nc


def _get_program():
    key = ("prog", str(MM_DT), SCHR_C)
    if key not in _PROG_CACHE:
        nc = bacc.Bacc("TRN2", target_bir_lowering=False, debug=False)
        _build_kernel(nc)
        nc.compile()
        _PROG_CACHE[key] = nc
    return _PROG_CACHE[key]


def _host_prep(x1, x2, Wq, bq, Wk, bk, Wv, bv, Wo, bo,
               lq1, lk1, lq2, lk2, gn_w, gn_b):
    f32 = np.float32
    x1 = np.asarray(x1, f32)
    x2 = np.asarray(x2, f32)
    lam = (
        np.exp((np.asarray(lq1, f32) * np.asarray(lk1, f32)).sum(-1))
        - np.exp((np.asarray(lq2, f32) * np.asarray(lk2, f32)).sum(-1))
        + f32(LAMBDA_INIT)
    ).astype(f32)  # (H,)
    sc = f32(1.0 - LAMBDA_INIT)
    gw = (np.asarray(gn_w, f32) * sc).reshape(H, HEAD)
    gb = (np.asarray(gn_b, f32) * sc).reshape(H, HEAD)
    Wq, Wk, Wv, Wo = (np.asarray(w, f32) for w in (Wq, Wk, Wv, Wo))
    bq, bk, bv, bo = (np.asarray(v_, f32) for v_ in (bq, bk, bv, bo))

    mdt = mybir.dt.np(MM_DT)
    x1T = np.ascontiguousarray(x1.reshape(NT, DIM).T).astype(mdt)
    x2T = np.ascontiguousarray(x2.reshape(NT, DIM).T).astype(mdt)
    vones_arr = np.zeros((128, 32, 65), mdt)
    vones_arr[:, :, 0:2] = 1.0
    ident_arr = np.eye(128, dtype=mdt)
    gnc_arr = np.ascontiguousarray(
        np.broadcast_to(
            np.array([[0x5F3759DF, 1]], np.uint32), (128, 2)
        )
    )

    in_maps = []
    for c in range(NCORES):
        dlo, dhi = c * DC, (c + 1) * DC
        h0 = c * HPC
        in_maps.append(
            {
                "x1T": x1T,
                "x2T": x2T,
                "wqT": np.ascontiguousarray(Wq[dlo:dhi, :].T).astype(mdt),
                "wkT": np.ascontiguousarray(Wk[dlo:dhi, :].T).astype(mdt),
                "wvT": np.ascontiguousarray(Wv[dlo:dhi, :].T).astype(mdt),
                "woT": np.ascontiguousarray(Wo[:, dlo:dhi].T).astype(mdt),
                "bqv": np.ascontiguousarray(bq[dlo:dhi].reshape(DC, 1)),
                "bkv": np.ascontiguousarray(bk[dlo:dhi].reshape(DC, 1)),
                "bvv": np.ascontiguousarray(bv[dlo:dhi].reshape(DC, 1)),
                "lamn": np.ascontiguousarray(
                    np.broadcast_to((-lam[h0 : h0 + HPC])[None, :], (128, HPC))
                ),
                "vones": vones_arr,
                "identr": ident_arr,
                "gnc": gnc_arr,
                "gwv": np.ascontiguousarray(gw[h0 : h0 + HPC].reshape(DC, 1)),
                "gbv": np.ascontiguousarray(gb[h0 : h0 + HPC].reshape(DC, 1)),
            }
        )

    def finish(results):
        o1 = np.zeros((NT, DIM), np.float64)
        o2 = np.zeros((NT, DIM), np.float64)
        for r in results:
            o1 += r["out1p"].astype(np.float64)
            o2 += r["out2p"].astype(np.float64)
        o1 = (o1 + bo).astype(f32).reshape(B, N, DIM)
        o2 = (o2 + bo).astype(f32).reshape(B, N, DIM)
        return o1, o2

    return in_maps, finish


def kernel(x1, x2, Wq, bq, Wk, bk, Wv, bv, Wo, bo,
           lq1, lk1, lq2, lk2, gn_w, gn_b):
    global LAST_EXEC_NS
    in_maps, finish = _host_prep(
        x1, x2, Wq, bq, Wk, bk, Wv, bv, Wo, bo,
        lq1, lk1, lq2, lk2, gn_w, gn_b,
    )
    nc = _get_program()
    trace = os.environ.get("BASS_KERNEL_TRACE", "0") == "1"
    res = run_bass_kernel_spmd(
        nc, in_maps, core_ids=list(range(NCORES)), trace=trace
    )
    LAST_EXEC_NS = res.exec_time_ns
    return finish(res.results)


# revision 29
# speedup vs baseline: 1.0084x; 1.0084x over previous
"""Differential cross-attention Bass kernel for 8 Trainium2 NeuronCores.

Sharding: heads are split across cores (2 of 16 heads per core). Each core
computes Q/K/V projections for its head slice, both N x N differential score
maps for its (batch, head) units, softmax (no max-subtraction; scores are
O(1) so exp is safe), attn = a1 - lam*a2, GroupNorm per (b, h), and a partial
output projection against its 128-column slice of Wo. The host sums the 8
partial outputs and adds the output bias.

Performance-critical design notes (v2):
- All big matmuls are fp16 with FULL k=128 contraction (HAM clock governor
  ignores <128-row matmuls). K stays dim-major as lhsT; Q is stored as four
  zero-masked variants so every score matmul is a standard 128x128 matmul.
- The exp stream (67M PSUM f32 elements/core) is THE co-bottleneck with the
  PE: it is split across BOTH ScalarE (exact spline exp, map1 columns plus a
  tunable slice of map2) and the DVE (map2 columns via a one-instruction
  Schraudolph: i16 = round(s*A + B), bits reinterpreted as fp16; the e^K
  range-shift and the sawtooth's mean cancel in the softmax division, and
  the remaining +-2.9% sawtooth error only enters scaled by lambda~0.36).
- All staging after PSUM is fp16: o12 is evacuated by ScalarE as fp16, the
  per-block transposes and tail transposes run at 1 cyc/row instead of 2,
  and the combine/affine DVE ops hit the 2x_1P packed mode. Partial outputs
  DMA to DRAM as fp16 (halves the output traffic; host sums in f64).
- Per-block combine: ONE batched reciprocal over all 8 (sub, map)
  denominators (strided AP into the single fp16 transpose tile), then
  ts+stt per sub. GroupNorm stats stay off PE/ACT (GpSimd all-reduce, DVE
  bit-trick rsqrt), computed per-head right after the head's blocks.
- Unit tails are split in half (h0 transposes+affine | h1 + Wo + output) and
  popped at two points of the NEXT unit; the LAST unit pops its h0 half
  during its own h1 blocks, shrinking the end-of-kernel half-clock window.
- Projection phase: x/wq DMAs are issued first, the q-variant zero-fills run
  on the idle GpSimd engine, and PSUM evacuation is split between ScalarE
  (Identity with per-partition bias AP) and the DVE so neither engine gates
  the x-chunk pipeline.
"""

import os
import sys
from contextlib import ExitStack

import numpy as np

for _p in ("/opt/trn_rl_repo", "/opt/pypackages"):
    if os.path.isdir(_p) and _p not in sys.path:
        sys.path.append(_p)

import concourse.bass as bass
import concourse.bass_isa as bass_isa
import concourse.tile as tile
from concourse import bacc, mybir
from concourse.bass_utils import run_bass_kernel_spmd

# ---- problem constants (hardcoded per contest contract) ----
B, N, DIM, H, HEAD, HALF = 2, 2048, 1024, 16, 64, 32
SCALE = HALF ** -0.5
LAMBDA_INIT = 0.8 - 0.6 * float(np.exp(-0.3 * (2 - 1)))
EPS = 1e-5
NCORES = 8
HPC = H // NCORES          # 2 heads per core
DC = HPC * HEAD            # 128 feature dims per core
NT = B * N                 # 4096 tokens
F32 = mybir.dt.float32
F16 = mybir.dt.float16
I16 = mybir.dt.int16

_DTMAP = {
    "float16": mybir.dt.float16,
    "bfloat16": mybir.dt.bfloat16,
    "float32r": mybir.dt.float32r,
    "float32": mybir.dt.float32,
}
MM_DT = _DTMAP[os.environ.get("BASS_MM_DT", "float16")]

# exp split: ACT handles map1 (cols 0:512 of each score tile) exactly; the
# DVE handles map2 (cols 512:1024) via Schraudolph. Separate p1/p2 tiles so
# the two engines run concurrently (a shared tile serializes the writers).
# Schraudolph fp16 constants: bits = v*SA + SB, v = s*SCALE (folded into SA)
SHIFT_K = 1.0              # extra e^K factor, cancels per-column in softmax
# (K=1: max fp16-staged denominator ~22k on the actual data, 3x margin;
#  negative scores stay out of fp16-subnormal territory down to v=-9.9)
SCHR_C = float(os.environ.get("BASS_SCHR_C", "-0.0434"))
SA = float(1024.0 / np.log(2.0)) * SCALE
SB = float((15.0 + SCHR_C) * 1024.0 + SHIFT_K * 1024.0 / np.log(2.0))

LAST_EXEC_NS = None
_PROG_CACHE = {}


def _build_kernel(nc):
    AF = mybir.ActivationFunctionType
    ALU = mybir.AluOpType
    AX = mybir.AxisListType

    x1T = nc.dram_tensor("x1T", (DIM, NT), MM_DT, kind="ExternalInput").ap()
    x2T = nc.dram_tensor("x2T", (DIM, NT), MM_DT, kind="ExternalInput").ap()
    wqT = nc.dram_tensor("wqT", (DIM, DC), MM_DT, kind="ExternalInput").ap()
    wkT = nc.dram_tensor("wkT", (DIM, DC), MM_DT, kind="ExternalInput").ap()
    wvT = nc.dram_tensor("wvT", (DIM, DC), MM_DT, kind="ExternalInput").ap()
    woT = nc.dram_tensor("woT", (DC, DIM), MM_DT, kind="ExternalInput").ap()
    bqv = nc.dram_tensor("bqv", (DC, 1), F32, kind="ExternalInput").ap()
    bkv = nc.dram_tensor("bkv", (DC, 1), F32, kind="ExternalInput").ap()
    bvv = nc.dram_tensor("bvv", (DC, 1), F32, kind="ExternalInput").ap()
    lamn = nc.dram_tensor("lamn", (128, HPC), F32, kind="ExternalInput").ap()
    vones = nc.dram_tensor("vones", (128, 32, 65), MM_DT, kind="ExternalInput").ap()
    identr = nc.dram_tensor("identr", (128, 128), MM_DT, kind="ExternalInput").ap()
    gwv = nc.dram_tensor("gwv", (DC, 1), F32, kind="ExternalInput").ap()
    gbv = nc.dram_tensor("gbv", (DC, 1), F32, kind="ExternalInput").ap()
    gnc = nc.dram_tensor(
        "gnc", (128, 2), mybir.dt.uint32, kind="ExternalInput"
    ).ap()
    out1p = nc.dram_tensor("out1p", (NT, DIM), F16, kind="ExternalOutput").ap()
    out2p = nc.dram_tensor("out2p", (NT, DIM), F16, kind="ExternalOutput").ap()

    with tile.TileContext(nc) as tc, ExitStack() as top:
        consts = top.enter_context(tc.tile_pool(name="consts", bufs=1))
        qkpool = top.enter_context(tc.tile_pool(name="qkpool", bufs=1))
        vpool = top.enter_context(tc.tile_pool(name="vpool", bufs=1))

        # ---- constants; wq + the first x chunk DMA first so the first
        # projection matmul starts as early as possible
        wq_t = consts.tile([128, 8, DC], MM_DT, tag="wq")
        wk_t = consts.tile([128, 8, DC], MM_DT, tag="wk")
        wv_t = consts.tile([128, 8, DC], MM_DT, tag="wv")
        nc.sync.dma_start(wq_t, wqT.rearrange("(kc p) d -> p kc d", p=128))
        x1Tr = x1T.rearrange("(kc p) t -> p kc t", p=128)
        x2Tr = x2T.rearrange("(kc p) t -> p kc t", p=128)
        xpre = consts.tile([128, 8, 1024], MM_DT, tag="xpre")
        nc.sync.dma_start(xpre, x1Tr[:, :, 0:1024])
        nc.sync.dma_start(wk_t, wkT.rearrange("(kc p) d -> p kc d", p=128))
        nc.sync.dma_start(wv_t, wvT.rearrange("(kc p) d -> p kc d", p=128))
        bq_t = consts.tile([DC, 1], F32, tag="bq")
        bk_t = consts.tile([DC, 1], F32, tag="bk")
        bv_t = consts.tile([DC, 1], F32, tag="bv")
        nc.sync.dma_start(bq_t, bqv)
        nc.sync.dma_start(bk_t, bkv)
        nc.sync.dma_start(bv_t, bvv)
        identr_t = consts.tile([128, 128], MM_DT, tag="identr")
        nc.sync.dma_start(identr_t, identr)
        wo_t = consts.tile([DC, DIM], MM_DT, tag="wo")
        nc.sync.dma_start(wo_t, woT)
        lam_t = consts.tile([128, HPC], F32, tag="lam")
        nc.sync.dma_start(lam_t, lamn)
        gw_t = consts.tile([DC, 1], F32, tag="gw")
        gb_t = consts.tile([DC, 1], F32, tag="gb")
        nc.sync.dma_start(gw_t, gwv)
        nc.sync.dma_start(gb_t, gbv)
        # uint32 constants for the DVE-only rsqrt: [0x5F3759DF magic, 1]
        gnc_t = consts.tile([128, 2], mybir.dt.uint32, tag="gnc")
        nc.sync.dma_start(gnc_t, gnc)

        # K dim-major; Q as 4 zero-masked variants per tensor (head x half)
        k1_t = qkpool.tile([128, NT], MM_DT, tag="k1")
        k2_t = qkpool.tile([128, NT], MM_DT, tag="k2")
        q1_v = [qkpool.tile([128, 2, NT], MM_DT, name=f"q1v{i}", tag=f"q1v{i}")
                for i in range(HPC)]
        q2_v = [qkpool.tile([128, 2, NT], MM_DT, name=f"q2v{i}", tag=f"q2v{i}")
                for i in range(HPC)]
        # variant zero-fill on the otherwise-idle GpSimd engine (keeps the
        # 4x ~7us memsets off the DVE's critical path during warmup)
        for v in q1_v + q2_v:
            nc.gpsimd.memset(v, 0.0)
        # V token-major: (tok 128, chunk 32, [64 h0 | 1 | 64 h1 | 1 | 63 pad])
        v1_t = vpool.tile([128, 32, 193], MM_DT, tag="v1")
        v2_t = vpool.tile([128, 32, 193], MM_DT, tag="v2")
        # ================= phase P: projections =================
        with ExitStack() as ph:
            xin = ph.enter_context(tc.tile_pool(name="xin", bufs=3))
            pqk = ph.enter_context(tc.tile_pool(name="pqk", bufs=3, space="PSUM"))
            pv = ph.enter_context(tc.tile_pool(name="pv", bufs=2, space="PSUM"))
            for xTr, qv, kd, vd in (
                (x1Tr, q1_v, k1_t, v1_t), (x2Tr, q2_v, k2_t, v2_t)
            ):
                for tcn in range(4):
                    ts0 = tcn * 1024
                    if xTr is x1Tr and tcn == 0:
                        xt = xpre  # prefetched before the consts DMAs
                    else:
                        xt = xin.tile([128, 8, 1024], MM_DT, tag="x")
                        nc.sync.dma_start(xt, xTr[:, :, ts0 : ts0 + 1024])
                    vstage = xin.tile([128, 1024], MM_DT, tag="vs")
                    for wt, bt, dst in (
                        (wq_t, bq_t, None), (wk_t, bk_t, kd), (wv_t, bv_t, vstage)
                    ):
                        ps = pqk.tile([128, 1024], F32, tag="qk")
                        for kc in range(8):
                            for jh in range(2):
                                nc.tensor.matmul(
                                    ps[:, jh * 512 : (jh + 1) * 512],
                                    lhsT=wt[:, kc, :],
                                    rhs=xt[:, kc, jh * 512 : (jh + 1) * 512],
                                    start=(kc == 0),
                                    stop=(kc == 7),
                                )
                        if dst is None:
                            # Q: scatter rows into the zero-masked head pairs
                            # (split ACT/DVE so neither engine gates the loop)
                            for h in range(HPC):
                                for j in range(2):
                                    hs = slice(h * 64 + j * 32, h * 64 + j * 32 + 32)
                                    dstap = qv[h][hs, j, ts0 : ts0 + 1024]
                                    if j == 0:
                                        nc.scalar.activation(
                                            dstap, ps[hs, :], AF.Identity,
                                            bias=bt[hs, :],
                                        )
                                    else:
                                        nc.vector.tensor_scalar(
                                            dstap, ps[hs, :], bt[hs, :], None,
                                            ALU.add,
                                        )
                        else:
                            out_ap = (
                                dst if dst is vstage else dst[:, ts0 : ts0 + 1024]
                            )
                            nc.scalar.activation(
                                out_ap, ps, AF.Identity, bias=bt
                            )
                    # transpose V chunk to token-major and scatter into V tile
                    for sc in range(8):
                        tp = pv.tile([128, 128], MM_DT, tag="v")
                        nc.tensor.transpose(
                            tp, vstage[:, sc * 128 : (sc + 1) * 128], identr_t
                        )
                        sg = tcn * 8 + sc
                        nc.vector.tensor_copy(vd[:, sg, 0:64], tp[:, 0:64])
                        nc.vector.tensor_copy(vd[:, sg, 65:129], tp[:, 64:128])
            for vd in (v1_t, v2_t):
                nc.sync.dma_start(vd[:, :, 64:65], vones[:, :, 0:1])
                nc.sync.dma_start(vd[:, :, 129:130], vones[:, :, 1:2])
                nc.sync.dma_start(vd[:, :, 130:193], vones[:, :, 2:65])
        # ================= phase A: attention =================
        with ExitStack() as ph:
            pp = ph.enter_context(tc.tile_pool(name="pp", bufs=6))
            osb = ph.enter_context(tc.tile_pool(name="osb", bufs=2))
            ocomb = ph.enter_context(tc.tile_pool(name="ocomb", bufs=2))
            otp = ph.enter_context(tc.tile_pool(name="otp", bufs=2))
            wos = ph.enter_context(tc.tile_pool(name="wos", bufs=4))
            small = ph.enter_context(tc.tile_pool(name="small", bufs=8))
            ps_s = ph.enter_context(tc.tile_pool(name="ps_s", bufs=2, space="PSUM"))
            ps_o = ph.enter_context(tc.tile_pool(name="ps_o", bufs=1, space="PSUM"))
            ps_t = ph.enter_context(tc.tile_pool(name="ps_t", bufs=2, space="PSUM"))

            # pending tails: (T1, T2, T3) closures per unit. T1 handles subs
            # 0..7 (transpose+affine), T2 subs 8..15, T3 the Wo projection +
            # output DMA. Non-last units pop T1 at (h0,qc1) and T2+T3 at
            # (h0,qc3) of the NEXT unit. The LAST unit runs its own T1 at
            # (h1,qc2) with the h0 affine rows only (h1 stats aren't ready),
            # and finishes the rest at the very end. All transposes are full
            # 128-partition ops (64-row ops downclock the PE via HAM).
            pending = []

            def make_tails(ot, ocu, sdv, tdv, b, outp):
                def subs_pass(s0, s1, rows):
                    for sub in range(s0, s1):
                        tb = ps_t.tile([128, 128], MM_DT, tag="t")
                        nc.tensor.transpose(tb, ocu[:, sub, :], identr_t)
                        nc.vector.tensor_scalar(
                            ot[rows, sub * 128 : (sub + 1) * 128],
                            tb[rows, :], sdv[rows, :], tdv[rows, :],
                            ALU.mult, ALU.add,
                        )

                def T1():
                    subs_pass(0, 8, slice(0, 128))

                def T2():
                    subs_pass(8, 16, slice(0, 128))

                def T_endgame():
                    # last unit only: all 16 transposes run right after the
                    # final combine (they don't need GN stats), dummy
                    # matmuls keep the PE/HAM hot while the GN chain drains,
                    # then the affines release and Wo runs at full clock.
                    tbig = [ps_t.tile([128, 8, 128], MM_DT,
                                      name=f"tbig{i}", tag="t")
                            for i in range(2)]
                    for sub in range(16):
                        nc.tensor.transpose(
                            tbig[sub // 8][:, sub % 8, :], ocu[:, sub, :],
                            identr_t,
                        )
                    for dk in range(16):
                        sd = ps_s.tile([128, 1024], F32, tag="s")
                        nc.tensor.matmul(
                            sd[:, 0:512],
                            lhsT=k1_t[:, 0:128],
                            rhs=k2_t[:, 0:512],
                            start=True,
                            stop=True,
                        )
                    for sub in range(16):
                        nc.vector.tensor_scalar(
                            ot[:, sub * 128 : (sub + 1) * 128],
                            tbig[sub // 8][:, sub % 8, :],
                            sdv, tdv, ALU.mult, ALU.add,
                        )

                def T3_piece(p):
                    def run():
                        for tl in range(4 * p, 4 * p + 4):
                            for nc2 in range(2):
                                wp = ps_t.tile([128, 512], F32, tag="t")
                                nc.tensor.matmul(
                                    wp,
                                    lhsT=ot[:, tl * 128 : (tl + 1) * 128],
                                    rhs=wo_t[:, nc2 * 512 : (nc2 + 1) * 512],
                                    start=True,
                                    stop=True,
                                )
                                ws = wos.tile([128, 512], F16, tag="ws")
                                # alternate evacuation engine to balance load
                                if (tl * 2 + nc2) % 2 == 0:
                                    nc.scalar.activation(ws, wp, AF.Copy)
                                else:
                                    nc.vector.tensor_copy(ws, wp)
                                nc.sync.dma_start(
                                    outp[
                                        b * N + tl * 128 : b * N + (tl + 1) * 128,
                                        nc2 * 512 : (nc2 + 1) * 512,
                                    ],
                                    ws,
                                )
                    return run
                return (T1, T_endgame, T2,
                        [T3_piece(p) for p in range(4)])

            units = [(b, a) for b in range(B) for a in range(2)]
            for ui, (b, a) in enumerate(units):
                last = ui == len(units) - 1
                qv = q1_v if a == 0 else q2_v
                kt = k2_t if a == 0 else k1_t
                vt = v2_t if a == 0 else v1_t
                outp = out1p if a == 0 else out2p
                ot = otp.tile([128, N], MM_DT, tag="ot")
                # combined, normalized attn output for BOTH heads:
                # (q 128, sub 16, [h0 64 | h1 64]) in fp16
                ocu = ocomb.tile([128, 16, 128], F16, tag="oc")
                sdv = small.tile([128, 1], F32, tag="sdv")
                tdv = small.tile([128, 1], F32, tag="tdv")
                my_tails = make_tails(ot, ocu, sdv, tdv, b, outp)
                for h in range(HPC):
                    hb = h * 64
                    for qc in range(4):
                        q0 = b * N + qc * 512
                        o12 = ps_o.tile([128, 1024], F32, tag="o")
                        va0 = h * 65  # [0:65]=h0 dims+ones, [65:130]=h1

                        # software-pipelined: S(kc) scores -> exp(kc) ->
                        # @V(kc-1), so PE never stalls on the exp engines.
                        # p1 (map1, exact ACT exp) and p2 (map2, DVE
                        # Schraudolph) are separate tiles so the two exp
                        # engines run concurrently.
                        p1tiles = [None] * 16
                        p2tiles = [None] * 16

                        def emit_scores(kc):
                            k0 = b * N + kc * 128
                            s = ps_s.tile([128, 1024], F32, tag="s")
                            # both halves against the same K slice;
                            # each matmul stays within one PSUM bank.
                            for j in range(2):
                                nc.tensor.matmul(
                                    s[:, j * 512 : (j + 1) * 512],
                                    lhsT=kt[:, k0 : k0 + 128],
                                    rhs=qv[h][:, j, q0 : q0 + 512],
                                    start=True,
                                    stop=True,
                                )
                            p1 = pp.tile([128, 512], MM_DT, tag="p1")
                            p2 = pp.tile([128, 512], MM_DT, tag="p2")
                            nc.scalar.activation(
                                p1, s[:, 0:512], AF.Exp, scale=SCALE,
                            )
                            nc.vector.tensor_scalar(
                                p2.bitcast(I16),
                                s[:, 512:1024],
                                SA, SB, ALU.mult, ALU.add,
                            )
                            p1tiles[kc] = p1
                            p2tiles[kc] = p2

                        def emit_av(kc):
                            # 65-wide lhsT (V dims + ones row): only rows
                            # 0:65 of o12 are read downstream, and narrower
                            # stationaries halve the weight-load time. The
                            # HAM clock keys on contraction rows (128 here).
                            vs = b * 16 + kc
                            for j, pt in ((0, p1tiles[kc]), (1, p2tiles[kc])):
                                nc.tensor.matmul(
                                    o12[0:65, j * 512 : (j + 1) * 512],
                                    lhsT=vt[:, vs, va0 : va0 + 65],
                                    rhs=pt,
                                    start=(kc == 0),
                                    stop=(kc == 15),
                                    skip_group_check=True,
                                )

                        STAG = 4
                        for kc in range(STAG):
                            emit_scores(kc)
                        for kc in range(STAG, 16):
                            emit_scores(kc)
                            emit_av(kc - STAG)
                        for kc in range(16 - STAG, 16):
                            emit_av(kc)
                        # interleave deferred tail pieces with the stream:
                        # T1 / T2 at (h0,qc1) / (h0,qc3); Wo in 4 pieces
                        # across the h1 blocks (a monolithic Wo burst paces
                        # the PE at PSUM-evacuation speed and lets HAM
                        # downclock).
                        if pending:
                            pt = pending[0]
                            if h == 0 and qc == 1:
                                pt[0]()              # prev unit T1
                            if h == 0 and qc == 3:
                                pt[2]()              # prev unit T2
                            if h == 1:
                                pt[3][qc]()          # prev unit T3 piece
                                if qc == 3:
                                    pending.pop(0)
                        # evacuate o12 PSUM -> fp16 staging on ACT (1 op)
                        os_t = osb.tile([65, 1024], F16, tag="os")
                        nc.scalar.activation(os_t, o12[0:65, :], AF.Copy)
                        # transpose to (q, [sub, 65]) -- single fp16 PSUM
                        # bank, 68-col stride keeps 8B alignment
                        t12 = ps_t.tile([128, 8, 68], F16, tag="t")
                        for i in range(8):
                            nc.tensor.transpose(
                                t12[:, i, 0:65],
                                os_t[:, i * 128 : (i + 1) * 128],
                                identr_t[0:65, 0:65],
                            )
                        # batched combine: one reciprocal for all 8 denoms.
                        # The fp16->f32 copy is tiny; feeding fp16 into
                        # reciprocal directly makes walrus use a low-precision
                        # fp16 divide path (costs ~2x final accuracy).
                        den = small.tile([128, 8], F32, tag="den")
                        nc.vector.tensor_copy(den, t12[:, :, 64:65])
                        rec = small.tile([128, 8], F32, tag="rec")
                        nc.vector.reciprocal(rec, den)
                        s2p = small.tile([128, 4], F32, tag="s2p")
                        nc.vector.tensor_scalar(
                            s2p, rec[:, 4:8], lam_t[:, h : h + 1], None,
                            ALU.mult,
                        )
                        for i in range(4):
                            sidx = qc * 4 + i
                            tmp = small.tile([128, 64], F16, tag="tmp")
                            nc.vector.tensor_scalar(
                                tmp, t12[:, i, 0:64], rec[:, i : i + 1], None,
                                ALU.mult,
                            )
                            # ocu = (O2 * s2p) + tmp   (s2p = -lam / sum2)
                            nc.vector.scalar_tensor_tensor(
                                ocu[:, sidx, hb : hb + 64],
                                t12[:, 4 + i, 0:64],
                                s2p[:, i : i + 1],
                                tmp,
                                ALU.mult,
                                ALU.add,
                            )
                        if last and h == 1 and qc == 3:
                            # bridge the combine gap so HAM stays at full
                            # clock into the endgame
                            for dk in range(8):
                                sd = ps_s.tile([128, 1024], F32, tag="s")
                                nc.tensor.matmul(
                                    sd[:, 0:512],
                                    lhsT=kt[:, b * N : b * N + 128],
                                    rhs=qv[h][:, 0, q0 : q0 + 512],
                                    start=True,
                                    stop=True,
                                )
                    # ---- GroupNorm stats for head h: free-dim sums ride
                    # along ACT activations via accum_out (a DVE reduce on
                    # fp16 input inserts big CAST instructions), then a
                    # GpSimd partition all-reduce + DVE bit-trick rsqrt.
                    st = small.tile([128, 2], F32, tag="st")
                    sq = ocomb.tile([128, 16, 64], F16, tag="sq")
                    nc.scalar.activation(
                        sq, ocu[:, :, hb : hb + 64], AF.Identity,
                        accum_out=st[:, 0:1],
                    )
                    nc.scalar.activation(
                        sq, ocu[:, :, hb : hb + 64], AF.Square,
                        accum_out=st[:, 1:2],
                    )
                    red = small.tile([128, 2], F32, tag="red")
                    nc.gpsimd.partition_all_reduce(
                        red, st, 128, bass_isa.ReduceOp.add
                    )
                    mr = small.tile([128, 2], F32, tag="mr")
                    nc.vector.tensor_scalar(
                        mr, red, 1.0 / (N * HEAD), None, ALU.mult
                    )
                    m2 = small.tile([128, 1], F32, tag="m2")
                    nc.vector.tensor_tensor(
                        m2, mr[:, 0:1], mr[:, 0:1], ALU.mult
                    )
                    var = small.tile([128, 1], F32, tag="var")
                    nc.vector.tensor_sub(var, mr[:, 1:2], m2)
                    # rstd via DVE-only bit-trick rsqrt (seed + 2 Newton)
                    hs = slice(h * 64, h * 64 + 64)
                    veps = small.tile([128, 1], F32, tag="veps")
                    nc.vector.tensor_scalar(veps, var, EPS, None, ALU.add)
                    vsh = small.tile([128, 1], mybir.dt.uint32, tag="vsh")
                    nc.vector.tensor_scalar(
                        vsh, veps.bitcast(mybir.dt.uint32),
                        gnc_t[:, 1:2], None, ALU.logical_shift_right,
                    )
                    y0i = small.tile([128, 1], mybir.dt.uint32, tag="y0i")
                    nc.vector.tensor_tensor(
                        y0i, gnc_t[:, 0:1], vsh, ALU.subtract
                    )
                    vh = small.tile([128, 1], F32, tag="vh")
                    nc.vector.tensor_scalar(vh, veps, 0.5, None, ALU.mult)
                    y = y0i.bitcast(F32)
                    for _ in range(2):  # Newton: y*(1.5-vh*y^2)
                        t1n = small.tile([128, 1], F32, tag="nt1")
                        nc.vector.tensor_tensor(t1n, y, y, ALU.mult)
                        t2n = small.tile([128, 1], F32, tag="nt2")
                        nc.vector.tensor_tensor(t2n, vh, t1n, ALU.mult)
                        t3n = small.tile([128, 1], F32, tag="nt3")
                        nc.vector.tensor_scalar(
                            t3n, t2n, 1.5, -1.0, ALU.subtract, ALU.mult
                        )
                        yn = small.tile([128, 1], F32, tag="yn")
                        nc.vector.tensor_tensor(yn, y, t3n, ALU.mult)
                        y = yn
                    tmp1 = small.tile([128, 1], F32, tag="tmp1")
                    nc.vector.tensor_tensor(
                        sdv[hs, :], gw_t[hs, :], y[hs, :], ALU.mult
                    )
                    nc.vector.tensor_tensor(
                        tmp1[hs, :], mr[hs, 0:1], sdv[hs, :], ALU.mult
                    )
                    nc.vector.tensor_sub(tdv[hs, :], gb_t[hs, :], tmp1[hs, :])
                if not last:
                    pending.append(my_tails)
            # final unit: transposes + dummies + affines, then Wo
            my_tails[1]()
            for piece in my_tails[3]:
                piece()
    return nc


def _get_program():
    key = ("prog", str(MM_DT), SCHR_C)
    if key not in _PROG_CACHE:
        nc = bacc.Bacc("TRN2", target_bir_lowering=False, debug=False)
        _build_kernel(nc)
        nc.compile()
        _PROG_CACHE[key] = nc
    return _PROG_CACHE[key]


def _host_prep(x1, x2, Wq, bq, Wk, bk, Wv, bv, Wo, bo,
               lq1, lk1, lq2, lk2, gn_w, gn_b):
    f32 = np.float32
    x1 = np.asarray(x1, f32)
    x2 = np.asarray(x2, f32)
    lam = (
        np.exp((np.asarray(lq1, f32) * np.asarray(lk1, f32)).sum(-1))
        - np.exp((np.asarray(lq2, f32) * np.asarray(lk2, f32)).sum(-1))
        + f32(LAMBDA_INIT)
    ).astype(f32)  # (H,)
    sc = f32(1.0 - LAMBDA_INIT)
    gw = (np.asarray(gn_w, f32) * sc).reshape(H, HEAD)
    gb = (np.asarray(gn_b, f32) * sc).reshape(H, HEAD)
    Wq, Wk, Wv, Wo = (np.asarray(w, f32) for w in (Wq, Wk, Wv, Wo))
    bq, bk, bv, bo = (np.asarray(v_, f32) for v_ in (bq, bk, bv, bo))

    mdt = mybir.dt.np(MM_DT)
    x1T = np.ascontiguousarray(x1.reshape(NT, DIM).T).astype(mdt)
    x2T = np.ascontiguousarray(x2.reshape(NT, DIM).T).astype(mdt)
    vones_arr = np.zeros((128, 32, 65), mdt)
    vones_arr[:, :, 0:2] = 1.0
    ident_arr = np.eye(128, dtype=mdt)
    gnc_arr = np.ascontiguousarray(
        np.broadcast_to(
            np.array([[0x5F3759DF, 1]], np.uint32), (128, 2)
        )
    )

    in_maps = []
    for c in range(NCORES):
        dlo, dhi = c * DC, (c + 1) * DC
        h0 = c * HPC
        in_maps.append(
            {
                "x1T": x1T,
                "x2T": x2T,
                "wqT": np.ascontiguousarray(Wq[dlo:dhi, :].T).astype(mdt),
                "wkT": np.ascontiguousarray(Wk[dlo:dhi, :].T).astype(mdt),
                "wvT": np.ascontiguousarray(Wv[dlo:dhi, :].T).astype(mdt),
                "woT": np.ascontiguousarray(Wo[:, dlo:dhi].T).astype(mdt),
                "bqv": np.ascontiguousarray(bq[dlo:dhi].reshape(DC, 1)),
                "bkv": np.ascontiguousarray(bk[dlo:dhi].reshape(DC, 1)),
                "bvv": np.ascontiguousarray(bv[dlo:dhi].reshape(DC, 1)),
                "lamn": np.ascontiguousarray(
                    np.broadcast_to((-lam[h0 : h0 + HPC])[None, :], (128, HPC))
                ),
                "vones": vones_arr,
                "identr": ident_arr,
                "gnc": gnc_arr,
                "gwv": np.ascontiguousarray(gw[h0 : h0 + HPC].reshape(DC, 1)),
                "gbv": np.ascontiguousarray(gb[h0 : h0 + HPC].reshape(DC, 1)),
            }
        )

    def finish(results):
        o1 = np.zeros((NT, DIM), np.float64)
        o2 = np.zeros((NT, DIM), np.float64)
        for r in results:
            o1 += r["out1p"].astype(np.float64)
            o2 += r["out2p"].astype(np.float64)
        o1 = (o1 + bo).astype(f32).reshape(B, N, DIM)
        o2 = (o2 + bo).astype(f32).reshape(B, N, DIM)
        return o1, o2

    return in_maps, finish


def kernel(x1, x2, Wq, bq, Wk, bk, Wv, bv, Wo, bo,
           lq1, lk1, lq2, lk2, gn_w, gn_b):
    global LAST_EXEC_NS
    in_maps, finish = _host_prep(
        x1, x2, Wq, bq, Wk, bk, Wv, bv, Wo, bo,
        lq1, lk1, lq2, lk2, gn_w, gn_b,
    )
    nc = _get_program()
    trace = os.environ.get("BASS_KERNEL_TRACE", "0") == "1"
    res = run_bass_kernel_spmd(
        nc, in_maps, core_ids=list(range(NCORES)), trace=trace
    )
    LAST_EXEC_NS = res.exec_time_ns
    return finish(res.results)


# revision 31
# speedup vs baseline: 1.0103x; 1.0019x over previous
"""Differential cross-attention Bass kernel for 8 Trainium2 NeuronCores.

Sharding: heads are split across cores (2 of 16 heads per core). Each core
computes Q/K/V projections for its head slice, both N x N differential score
maps for its (batch, head) units, softmax (no max-subtraction; scores are
O(1) so exp is safe), attn = a1 - lam*a2, GroupNorm per (b, h), and a partial
output projection against its 128-column slice of Wo. The host sums the 8
partial outputs and adds the output bias.

Performance-critical design notes (v2):
- All big matmuls are fp16 with FULL k=128 contraction (HAM clock governor
  ignores <128-row matmuls). K stays dim-major as lhsT; Q is stored as four
  zero-masked variants so every score matmul is a standard 128x128 matmul.
- The exp stream (67M PSUM f32 elements/core) is THE co-bottleneck with the
  PE: it is split across BOTH ScalarE (exact spline exp, map1 columns plus a
  tunable slice of map2) and the DVE (map2 columns via a one-instruction
  Schraudolph: i16 = round(s*A + B), bits reinterpreted as fp16; the e^K
  range-shift and the sawtooth's mean cancel in the softmax division, and
  the remaining +-2.9% sawtooth error only enters scaled by lambda~0.36).
- All staging after PSUM is fp16: o12 is evacuated by ScalarE as fp16, the
  per-block transposes and tail transposes run at 1 cyc/row instead of 2,
  and the combine/affine DVE ops hit the 2x_1P packed mode. Partial outputs
  DMA to DRAM as fp16 (halves the output traffic; host sums in f64).
- Per-block combine: ONE batched reciprocal over all 8 (sub, map)
  denominators (strided AP into the single fp16 transpose tile), then
  ts+stt per sub. GroupNorm stats stay off PE/ACT (GpSimd all-reduce, DVE
  bit-trick rsqrt), computed per-head right after the head's blocks.
- Unit tails are split in half (h0 transposes+affine | h1 + Wo + output) and
  popped at two points of the NEXT unit; the LAST unit pops its h0 half
  during its own h1 blocks, shrinking the end-of-kernel half-clock window.
- Projection phase: x/wq DMAs are issued first, the q-variant zero-fills run
  on the idle GpSimd engine, and PSUM evacuation is split between ScalarE
  (Identity with per-partition bias AP) and the DVE so neither engine gates
  the x-chunk pipeline.
"""

import os
import sys
from contextlib import ExitStack

import numpy as np

for _p in ("/opt/trn_rl_repo", "/opt/pypackages"):
    if os.path.isdir(_p) and _p not in sys.path:
        sys.path.append(_p)

import concourse.bass as bass
import concourse.bass_isa as bass_isa
import concourse.tile as tile
from concourse import bacc, mybir
from concourse.bass_utils import run_bass_kernel_spmd

# ---- problem constants (hardcoded per contest contract) ----
B, N, DIM, H, HEAD, HALF = 2, 2048, 1024, 16, 64, 32
SCALE = HALF ** -0.5
LAMBDA_INIT = 0.8 - 0.6 * float(np.exp(-0.3 * (2 - 1)))
EPS = 1e-5
NCORES = 8
HPC = H // NCORES          # 2 heads per core
DC = HPC * HEAD            # 128 feature dims per core
NT = B * N                 # 4096 tokens
F32 = mybir.dt.float32
F16 = mybir.dt.float16
I16 = mybir.dt.int16

_DTMAP = {
    "float16": mybir.dt.float16,
    "bfloat16": mybir.dt.bfloat16,
    "float32r": mybir.dt.float32r,
    "float32": mybir.dt.float32,
}
MM_DT = _DTMAP[os.environ.get("BASS_MM_DT", "float16")]

# exp split: ACT handles map1 (cols 0:512 of each score tile) exactly; the
# DVE handles map2 (cols 512:1024) via Schraudolph. Separate p1/p2 tiles so
# the two engines run concurrently (a shared tile serializes the writers).
# Schraudolph fp16 constants: bits = v*SA + SB, v = s*SCALE (folded into SA)
SHIFT_K = 1.0              # extra e^K factor, cancels per-column in softmax
# (K=1: max fp16-staged denominator ~22k on the actual data, 3x margin;
#  negative scores stay out of fp16-subnormal territory down to v=-9.9)
SCHR_C = float(os.environ.get("BASS_SCHR_C", "-0.0434"))
SA = float(1024.0 / np.log(2.0)) * SCALE
SB = float((15.0 + SCHR_C) * 1024.0 + SHIFT_K * 1024.0 / np.log(2.0))

LAST_EXEC_NS = None
_PROG_CACHE = {}


def _build_kernel(nc):
    AF = mybir.ActivationFunctionType
    ALU = mybir.AluOpType
    AX = mybir.AxisListType

    x1T = nc.dram_tensor("x1T", (DIM, NT), MM_DT, kind="ExternalInput").ap()
    x2T = nc.dram_tensor("x2T", (DIM, NT), MM_DT, kind="ExternalInput").ap()
    wqT = nc.dram_tensor("wqT", (DIM, DC), MM_DT, kind="ExternalInput").ap()
    wkT = nc.dram_tensor("wkT", (DIM, DC), MM_DT, kind="ExternalInput").ap()
    wvT = nc.dram_tensor("wvT", (DIM, DC), MM_DT, kind="ExternalInput").ap()
    woT = nc.dram_tensor("woT", (DC, DIM), MM_DT, kind="ExternalInput").ap()
    bqv = nc.dram_tensor("bqv", (DC, 1), F32, kind="ExternalInput").ap()
    bkv = nc.dram_tensor("bkv", (DC, 1), F32, kind="ExternalInput").ap()
    bvv = nc.dram_tensor("bvv", (DC, 1), F32, kind="ExternalInput").ap()
    lamn = nc.dram_tensor("lamn", (128, HPC), F32, kind="ExternalInput").ap()
    vones = nc.dram_tensor("vones", (128, 32, 65), MM_DT, kind="ExternalInput").ap()
    identr = nc.dram_tensor("identr", (128, 128), MM_DT, kind="ExternalInput").ap()
    gwv = nc.dram_tensor("gwv", (DC, 1), F32, kind="ExternalInput").ap()
    gbv = nc.dram_tensor("gbv", (DC, 1), F32, kind="ExternalInput").ap()
    gnc = nc.dram_tensor(
        "gnc", (128, 2), mybir.dt.uint32, kind="ExternalInput"
    ).ap()
    out1p = nc.dram_tensor("out1p", (NT, DIM), F16, kind="ExternalOutput").ap()
    out2p = nc.dram_tensor("out2p", (NT, DIM), F16, kind="ExternalOutput").ap()

    with tile.TileContext(nc) as tc, ExitStack() as top:
        consts = top.enter_context(tc.tile_pool(name="consts", bufs=1))
        qkpool = top.enter_context(tc.tile_pool(name="qkpool", bufs=1))
        vpool = top.enter_context(tc.tile_pool(name="vpool", bufs=1))

        # ---- constants; wq + the first x chunk DMA first so the first
        # projection matmul starts as early as possible
        wq_t = consts.tile([128, 8, DC], MM_DT, tag="wq")
        wk_t = consts.tile([128, 8, DC], MM_DT, tag="wk")
        wv_t = consts.tile([128, 8, DC], MM_DT, tag="wv")
        nc.sync.dma_start(wq_t, wqT.rearrange("(kc p) d -> p kc d", p=128))
        x1Tr = x1T.rearrange("(kc p) t -> p kc t", p=128)
        x2Tr = x2T.rearrange("(kc p) t -> p kc t", p=128)
        xpre = consts.tile([128, 8, 1024], MM_DT, tag="xpre")
        nc.sync.dma_start(xpre, x1Tr[:, :, 0:1024])
        nc.sync.dma_start(wk_t, wkT.rearrange("(kc p) d -> p kc d", p=128))
        nc.sync.dma_start(wv_t, wvT.rearrange("(kc p) d -> p kc d", p=128))
        bq_t = consts.tile([DC, 1], F32, tag="bq")
        bk_t = consts.tile([DC, 1], F32, tag="bk")
        bv_t = consts.tile([DC, 1], F32, tag="bv")
        nc.sync.dma_start(bq_t, bqv)
        nc.sync.dma_start(bk_t, bkv)
        nc.sync.dma_start(bv_t, bvv)
        identr_t = consts.tile([128, 128], MM_DT, tag="identr")
        nc.sync.dma_start(identr_t, identr)
        wo_t = consts.tile([DC, DIM], MM_DT, tag="wo")
        nc.sync.dma_start(wo_t, woT)
        lam_t = consts.tile([128, HPC], F32, tag="lam")
        nc.sync.dma_start(lam_t, lamn)
        gw_t = consts.tile([DC, 1], F32, tag="gw")
        gb_t = consts.tile([DC, 1], F32, tag="gb")
        nc.sync.dma_start(gw_t, gwv)
        nc.sync.dma_start(gb_t, gbv)
        # uint32 constants for the DVE-only rsqrt: [0x5F3759DF magic, 1]
        gnc_t = consts.tile([128, 2], mybir.dt.uint32, tag="gnc")
        nc.sync.dma_start(gnc_t, gnc)

        # K dim-major; Q as 4 zero-masked variants per tensor (head x half)
        k1_t = qkpool.tile([128, NT], MM_DT, tag="k1")
        k2_t = qkpool.tile([128, NT], MM_DT, tag="k2")
        q1_v = [qkpool.tile([128, 2, NT], MM_DT, name=f"q1v{i}", tag=f"q1v{i}")
                for i in range(HPC)]
        q2_v = [qkpool.tile([128, 2, NT], MM_DT, name=f"q2v{i}", tag=f"q2v{i}")
                for i in range(HPC)]
        # variant zero-fill on the otherwise-idle GpSimd engine (keeps the
        # 4x ~7us memsets off the DVE's critical path during warmup)
        for v in q1_v + q2_v:
            nc.gpsimd.memset(v, 0.0)
        # V token-major: (tok 128, chunk 32, [64 h0 | 1 | 64 h1 | 1 | 63 pad])
        v1_t = vpool.tile([128, 32, 193], MM_DT, tag="v1")
        v2_t = vpool.tile([128, 32, 193], MM_DT, tag="v2")
        # ================= phase P: projections =================
        with ExitStack() as ph:
            xin = ph.enter_context(tc.tile_pool(name="xin", bufs=3))
            pqk = ph.enter_context(tc.tile_pool(name="pqk", bufs=3, space="PSUM"))
            pv = ph.enter_context(tc.tile_pool(name="pv", bufs=2, space="PSUM"))
            for xTr, qv, kd, vd in (
                (x1Tr, q1_v, k1_t, v1_t), (x2Tr, q2_v, k2_t, v2_t)
            ):
                for tcn in range(4):
                    ts0 = tcn * 1024
                    if xTr is x1Tr and tcn == 0:
                        xt = xpre  # prefetched before the consts DMAs
                    else:
                        xt = xin.tile([128, 8, 1024], MM_DT, tag="x")
                        nc.sync.dma_start(xt, xTr[:, :, ts0 : ts0 + 1024])
                    vstage = xin.tile([128, 1024], MM_DT, tag="vs")
                    for wt, bt, dst in (
                        (wq_t, bq_t, None), (wk_t, bk_t, kd), (wv_t, bv_t, vstage)
                    ):
                        ps = pqk.tile([128, 1024], F32, tag="qk")
                        for kc in range(8):
                            for jh in range(2):
                                nc.tensor.matmul(
                                    ps[:, jh * 512 : (jh + 1) * 512],
                                    lhsT=wt[:, kc, :],
                                    rhs=xt[:, kc, jh * 512 : (jh + 1) * 512],
                                    start=(kc == 0),
                                    stop=(kc == 7),
                                )
                        if dst is None:
                            # Q: scatter rows into the zero-masked head pairs
                            # (split ACT/DVE so neither engine gates the loop)
                            for h in range(HPC):
                                for j in range(2):
                                    hs = slice(h * 64 + j * 32, h * 64 + j * 32 + 32)
                                    dstap = qv[h][hs, j, ts0 : ts0 + 1024]
                                    if j == 0:
                                        nc.scalar.activation(
                                            dstap, ps[hs, :], AF.Identity,
                                            bias=bt[hs, :],
                                        )
                                    else:
                                        nc.vector.tensor_scalar(
                                            dstap, ps[hs, :], bt[hs, :], None,
                                            ALU.add,
                                        )
                        else:
                            out_ap = (
                                dst if dst is vstage else dst[:, ts0 : ts0 + 1024]
                            )
                            nc.scalar.activation(
                                out_ap, ps, AF.Identity, bias=bt
                            )
                    # transpose V chunk to token-major and scatter into V tile
                    for sc in range(8):
                        tp = pv.tile([128, 128], MM_DT, tag="v")
                        nc.tensor.transpose(
                            tp, vstage[:, sc * 128 : (sc + 1) * 128], identr_t
                        )
                        sg = tcn * 8 + sc
                        nc.vector.tensor_copy(vd[:, sg, 0:64], tp[:, 0:64])
                        nc.vector.tensor_copy(vd[:, sg, 65:129], tp[:, 64:128])
            for vd in (v1_t, v2_t):
                nc.sync.dma_start(vd[:, :, 64:65], vones[:, :, 0:1])
                nc.sync.dma_start(vd[:, :, 129:130], vones[:, :, 1:2])
                nc.sync.dma_start(vd[:, :, 130:193], vones[:, :, 2:65])
        # ================= phase A: attention =================
        with ExitStack() as ph:
            pp = ph.enter_context(tc.tile_pool(name="pp", bufs=6))
            osb = ph.enter_context(tc.tile_pool(name="osb", bufs=2))
            ocomb = ph.enter_context(tc.tile_pool(name="ocomb", bufs=2))
            otp = ph.enter_context(tc.tile_pool(name="otp", bufs=2))
            wos = ph.enter_context(tc.tile_pool(name="wos", bufs=4))
            small = ph.enter_context(tc.tile_pool(name="small", bufs=8))
            ps_s = ph.enter_context(tc.tile_pool(name="ps_s", bufs=2, space="PSUM"))
            ps_o = ph.enter_context(tc.tile_pool(name="ps_o", bufs=1, space="PSUM"))
            ps_t = ph.enter_context(tc.tile_pool(name="ps_t", bufs=2, space="PSUM"))

            # pending tails: (T1, T2, T3) closures per unit. T1 handles subs
            # 0..7 (transpose+affine), T2 subs 8..15, T3 the Wo projection +
            # output DMA. Non-last units pop T1 at (h0,qc1) and T2+T3 at
            # (h0,qc3) of the NEXT unit. The LAST unit runs its own T1 at
            # (h1,qc2) with the h0 affine rows only (h1 stats aren't ready),
            # and finishes the rest at the very end. All transposes are full
            # 128-partition ops (64-row ops downclock the PE via HAM).
            pending = []

            def make_tails(ot, ocu, sdv, tdv, b, outp):
                def subs_pass(s0, s1, rows):
                    for sub in range(s0, s1):
                        tb = ps_t.tile([128, 128], MM_DT, tag="t")
                        nc.tensor.transpose(tb, ocu[:, sub, :], identr_t)
                        nc.vector.tensor_scalar(
                            ot[rows, sub * 128 : (sub + 1) * 128],
                            tb[rows, :], sdv[rows, :], tdv[rows, :],
                            ALU.mult, ALU.add,
                        )

                def T1():
                    subs_pass(0, 8, slice(0, 128))

                def T2():
                    subs_pass(8, 16, slice(0, 128))

                def T_endgame():
                    # last unit only: all 16 transposes run right after the
                    # final combine (they don't need GN stats), dummy
                    # matmuls keep the PE/HAM hot while the GN chain drains,
                    # then the affines release and Wo runs at full clock.
                    tbig = [ps_t.tile([128, 8, 128], MM_DT,
                                      name=f"tbig{i}", tag="t")
                            for i in range(2)]
                    for sub in range(16):
                        nc.tensor.transpose(
                            tbig[sub // 8][:, sub % 8, :], ocu[:, sub, :],
                            identr_t,
                        )
                    for dk in range(16):
                        sd = ps_s.tile([128, 1024], F32, tag="s")
                        nc.tensor.matmul(
                            sd[:, 0:512],
                            lhsT=k1_t[:, 0:128],
                            rhs=k2_t[:, 0:512],
                            start=True,
                            stop=True,
                        )
                    for sub in range(16):
                        # split affines across both engines (ACT Identity
                        # supports per-partition scale/bias APs)
                        dst = ot[:, sub * 128 : (sub + 1) * 128]
                        src = tbig[sub // 8][:, sub % 8, :]
                        if sub % 2 == 0:
                            nc.scalar.activation(
                                dst, src, AF.Identity, bias=tdv, scale=sdv
                            )
                        else:
                            nc.vector.tensor_scalar(
                                dst, src, sdv, tdv, ALU.mult, ALU.add
                            )

                def T3_piece(p):
                    def run():
                        for tl in range(4 * p, 4 * p + 4):
                            for nc2 in range(2):
                                wp = ps_t.tile([128, 512], F32, tag="t")
                                nc.tensor.matmul(
                                    wp,
                                    lhsT=ot[:, tl * 128 : (tl + 1) * 128],
                                    rhs=wo_t[:, nc2 * 512 : (nc2 + 1) * 512],
                                    start=True,
                                    stop=True,
                                )
                                ws = wos.tile([128, 512], F16, tag="ws")
                                # evacuate each wp in halves on BOTH engines
                                # concurrently: frees the 2-buf wp ring
                                # faster, which paces the whole Wo chain
                                nc.scalar.activation(
                                    ws[:, 0:256], wp[:, 0:256], AF.Copy
                                )
                                nc.vector.tensor_copy(
                                    ws[:, 256:512], wp[:, 256:512]
                                )
                                nc.sync.dma_start(
                                    outp[
                                        b * N + tl * 128 : b * N + (tl + 1) * 128,
                                        nc2 * 512 : (nc2 + 1) * 512,
                                    ],
                                    ws,
                                )
                    return run
                return (T1, T_endgame, T2,
                        [T3_piece(p) for p in range(4)])

            units = [(b, a) for b in range(B) for a in range(2)]
            for ui, (b, a) in enumerate(units):
                last = ui == len(units) - 1
                qv = q1_v if a == 0 else q2_v
                kt = k2_t if a == 0 else k1_t
                vt = v2_t if a == 0 else v1_t
                outp = out1p if a == 0 else out2p
                ot = otp.tile([128, N], MM_DT, tag="ot")
                # combined, normalized attn output for BOTH heads:
                # (q 128, sub 16, [h0 64 | h1 64]) in fp16
                ocu = ocomb.tile([128, 16, 128], F16, tag="oc")
                sdv = small.tile([128, 1], F32, tag="sdv")
                tdv = small.tile([128, 1], F32, tag="tdv")
                my_tails = make_tails(ot, ocu, sdv, tdv, b, outp)
                for h in range(HPC):
                    hb = h * 64
                    for qc in range(4):
                        q0 = b * N + qc * 512
                        o12 = ps_o.tile([128, 1024], F32, tag="o")
                        va0 = h * 65  # [0:65]=h0 dims+ones, [65:130]=h1

                        # software-pipelined: S(kc) scores -> exp(kc) ->
                        # @V(kc-1), so PE never stalls on the exp engines.
                        # p1 (map1, exact ACT exp) and p2 (map2, DVE
                        # Schraudolph) are separate tiles so the two exp
                        # engines run concurrently.
                        p1tiles = [None] * 16
                        p2tiles = [None] * 16

                        def emit_scores(kc):
                            k0 = b * N + kc * 128
                            s = ps_s.tile([128, 1024], F32, tag="s")
                            # both halves against the same K slice;
                            # each matmul stays within one PSUM bank.
                            for j in range(2):
                                nc.tensor.matmul(
                                    s[:, j * 512 : (j + 1) * 512],
                                    lhsT=kt[:, k0 : k0 + 128],
                                    rhs=qv[h][:, j, q0 : q0 + 512],
                                    start=True,
                                    stop=True,
                                )
                            p1 = pp.tile([128, 512], MM_DT, tag="p1")
                            p2 = pp.tile([128, 512], MM_DT, tag="p2")
                            nc.scalar.activation(
                                p1, s[:, 0:512], AF.Exp, scale=SCALE,
                            )
                            nc.vector.tensor_scalar(
                                p2.bitcast(I16),
                                s[:, 512:1024],
                                SA, SB, ALU.mult, ALU.add,
                            )
                            p1tiles[kc] = p1
                            p2tiles[kc] = p2

                        def emit_av(kc):
                            # 65-wide lhsT (V dims + ones row): only rows
                            # 0:65 of o12 are read downstream, and narrower
                            # stationaries halve the weight-load time. The
                            # HAM clock keys on contraction rows (128 here).
                            vs = b * 16 + kc
                            for j, pt in ((0, p1tiles[kc]), (1, p2tiles[kc])):
                                nc.tensor.matmul(
                                    o12[0:65, j * 512 : (j + 1) * 512],
                                    lhsT=vt[:, vs, va0 : va0 + 65],
                                    rhs=pt,
                                    start=(kc == 0),
                                    stop=(kc == 15),
                                    skip_group_check=True,
                                )

                        STAG = 4
                        for kc in range(STAG):
                            emit_scores(kc)
                        for kc in range(STAG, 16):
                            emit_scores(kc)
                            emit_av(kc - STAG)
                        for kc in range(16 - STAG, 16):
                            emit_av(kc)
                        # interleave deferred tail pieces with the stream:
                        # T1 / T2 at (h0,qc1) / (h0,qc3); Wo in 4 pieces
                        # across the h1 blocks (a monolithic Wo burst paces
                        # the PE at PSUM-evacuation speed and lets HAM
                        # downclock).
                        if pending:
                            pt = pending[0]
                            if h == 0 and qc == 1:
                                pt[0]()              # prev unit T1
                            if h == 0 and qc == 3:
                                pt[2]()              # prev unit T2
                            if h == 1:
                                pt[3][qc]()          # prev unit T3 piece
                                if qc == 3:
                                    pending.pop(0)
                        # evacuate o12 PSUM -> fp16 staging on ACT (1 op)
                        os_t = osb.tile([65, 1024], F16, tag="os")
                        nc.scalar.activation(os_t, o12[0:65, :], AF.Copy)
                        # transpose to (q, [sub, 65]) -- single fp16 PSUM
                        # bank, 68-col stride keeps 8B alignment
                        t12 = ps_t.tile([128, 8, 68], F16, tag="t")
                        for i in range(8):
                            nc.tensor.transpose(
                                t12[:, i, 0:65],
                                os_t[:, i * 128 : (i + 1) * 128],
                                identr_t[0:65, 0:65],
                            )
                        # batched combine: one reciprocal for all 8 denoms.
                        # The fp16->f32 copy is tiny; feeding fp16 into
                        # reciprocal directly makes walrus use a low-precision
                        # fp16 divide path (costs ~2x final accuracy).
                        den = small.tile([128, 8], F32, tag="den")
                        nc.vector.tensor_copy(den, t12[:, :, 64:65])
                        rec = small.tile([128, 8], F32, tag="rec")
                        nc.vector.reciprocal(rec, den)
                        s2p = small.tile([128, 4], F32, tag="s2p")
                        nc.vector.tensor_scalar(
                            s2p, rec[:, 4:8], lam_t[:, h : h + 1], None,
                            ALU.mult,
                        )
                        for i in range(4):
                            sidx = qc * 4 + i
                            tmp = small.tile([128, 64], F16, tag="tmp")
                            nc.vector.tensor_scalar(
                                tmp, t12[:, i, 0:64], rec[:, i : i + 1], None,
                                ALU.mult,
                            )
                            # ocu = (O2 * s2p) + tmp   (s2p = -lam / sum2)
                            nc.vector.scalar_tensor_tensor(
                                ocu[:, sidx, hb : hb + 64],
                                t12[:, 4 + i, 0:64],
                                s2p[:, i : i + 1],
                                tmp,
                                ALU.mult,
                                ALU.add,
                            )
                        if last and h == 1 and qc == 3:
                            # bridge the combine gap so HAM stays at full
                            # clock into the endgame
                            for dk in range(8):
                                sd = ps_s.tile([128, 1024], F32, tag="s")
                                nc.tensor.matmul(
                                    sd[:, 0:512],
                                    lhsT=kt[:, b * N : b * N + 128],
                                    rhs=qv[h][:, 0, q0 : q0 + 512],
                                    start=True,
                                    stop=True,
                                )
                    # ---- GroupNorm stats for head h: free-dim sums ride
                    # along ACT activations via accum_out (a DVE reduce on
                    # fp16 input inserts big CAST instructions), then a
                    # GpSimd partition all-reduce + DVE bit-trick rsqrt.
                    st = small.tile([128, 2], F32, tag="st")
                    sq = ocomb.tile([128, 16, 64], F16, tag="sq")
                    nc.scalar.activation(
                        sq, ocu[:, :, hb : hb + 64], AF.Identity,
                        accum_out=st[:, 0:1],
                    )
                    nc.scalar.activation(
                        sq, ocu[:, :, hb : hb + 64], AF.Square,
                        accum_out=st[:, 1:2],
                    )
                    red = small.tile([128, 2], F32, tag="red")
                    nc.gpsimd.partition_all_reduce(
                        red, st, 128, bass_isa.ReduceOp.add
                    )
                    mr = small.tile([128, 2], F32, tag="mr")
                    nc.vector.tensor_scalar(
                        mr, red, 1.0 / (N * HEAD), None, ALU.mult
                    )
                    m2 = small.tile([128, 1], F32, tag="m2")
                    nc.vector.tensor_tensor(
                        m2, mr[:, 0:1], mr[:, 0:1], ALU.mult
                    )
                    var = small.tile([128, 1], F32, tag="var")
                    nc.vector.tensor_sub(var, mr[:, 1:2], m2)
                    # rstd via DVE-only bit-trick rsqrt (seed + 2 Newton)
                    hs = slice(h * 64, h * 64 + 64)
                    veps = small.tile([128, 1], F32, tag="veps")
                    nc.vector.tensor_scalar(veps, var, EPS, None, ALU.add)
                    vsh = small.tile([128, 1], mybir.dt.uint32, tag="vsh")
                    nc.vector.tensor_scalar(
                        vsh, veps.bitcast(mybir.dt.uint32),
                        gnc_t[:, 1:2], None, ALU.logical_shift_right,
                    )
                    y0i = small.tile([128, 1], mybir.dt.uint32, tag="y0i")
                    nc.vector.tensor_tensor(
                        y0i, gnc_t[:, 0:1], vsh, ALU.subtract
                    )
                    vh = small.tile([128, 1], F32, tag="vh")
                    nc.vector.tensor_scalar(vh, veps, 0.5, None, ALU.mult)
                    y = y0i.bitcast(F32)
                    for _ in range(2):  # Newton: y*(1.5-vh*y^2)
                        t1n = small.tile([128, 1], F32, tag="nt1")
                        nc.vector.tensor_tensor(t1n, y, y, ALU.mult)
                        t2n = small.tile([128, 1], F32, tag="nt2")
                        nc.vector.tensor_tensor(t2n, vh, t1n, ALU.mult)
                        t3n = small.tile([128, 1], F32, tag="nt3")
                        nc.vector.tensor_scalar(
                            t3n, t2n, 1.5, -1.0, ALU.subtract, ALU.mult
                        )
                        yn = small.tile([128, 1], F32, tag="yn")
                        nc.vector.tensor_tensor(yn, y, t3n, ALU.mult)
                        y = yn
                    tmp1 = small.tile([128, 1], F32, tag="tmp1")
                    nc.vector.tensor_tensor(
                        sdv[hs, :], gw_t[hs, :], y[hs, :], ALU.mult
                    )
                    nc.vector.tensor_tensor(
                        tmp1[hs, :], mr[hs, 0:1], sdv[hs, :], ALU.mult
                    )
                    nc.vector.tensor_sub(tdv[hs, :], gb_t[hs, :], tmp1[hs, :])
                if not last:
                    pending.append(my_tails)
            # final unit: transposes + dummies + affines, then Wo
            my_tails[1]()
            for piece in my_tails[3]:
                piece()
    return nc


def _get_program():
    key = ("prog", str(MM_DT), SCHR_C)
    if key not in _PROG_CACHE:
        nc = bacc.Bacc("TRN2", target_bir_lowering=False, debug=False)
        _build_kernel(nc)
        nc.compile()
        _PROG_CACHE[key] = nc
    return _PROG_CACHE[key]


def _host_prep(x1, x2, Wq, bq, Wk, bk, Wv, bv, Wo, bo,
               lq1, lk1, lq2, lk2, gn_w, gn_b):
    f32 = np.float32
    x1 = np.asarray(x1, f32)
    x2 = np.asarray(x2, f32)
    lam = (
        np.exp((np.asarray(lq1, f32) * np.asarray(lk1, f32)).sum(-1))
        - np.exp((np.asarray(lq2, f32) * np.asarray(lk2, f32)).sum(-1))
        + f32(LAMBDA_INIT)
    ).astype(f32)  # (H,)
    sc = f32(1.0 - LAMBDA_INIT)
    gw = (np.asarray(gn_w, f32) * sc).reshape(H, HEAD)
    gb = (np.asarray(gn_b, f32) * sc).reshape(H, HEAD)
    Wq, Wk, Wv, Wo = (np.asarray(w, f32) for w in (Wq, Wk, Wv, Wo))
    bq, bk, bv, bo = (np.asarray(v_, f32) for v_ in (bq, bk, bv, bo))

    mdt = mybir.dt.np(MM_DT)
    x1T = np.ascontiguousarray(x1.reshape(NT, DIM).T).astype(mdt)
    x2T = np.ascontiguousarray(x2.reshape(NT, DIM).T).astype(mdt)
    vones_arr = np.zeros((128, 32, 65), mdt)
    vones_arr[:, :, 0:2] = 1.0
    ident_arr = np.eye(128, dtype=mdt)
    gnc_arr = np.ascontiguousarray(
        np.broadcast_to(
            np.array([[0x5F3759DF, 1]], np.uint32), (128, 2)
        )
    )

    in_maps = []
    for c in range(NCORES):
        dlo, dhi = c * DC, (c + 1) * DC
        h0 = c * HPC
        in_maps.append(
            {
                "x1T": x1T,
                "x2T": x2T,
                "wqT": np.ascontiguousarray(Wq[dlo:dhi, :].T).astype(mdt),
                "wkT": np.ascontiguousarray(Wk[dlo:dhi, :].T).astype(mdt),
                "wvT": np.ascontiguousarray(Wv[dlo:dhi, :].T).astype(mdt),
                "woT": np.ascontiguousarray(Wo[:, dlo:dhi].T).astype(mdt),
                "bqv": np.ascontiguousarray(bq[dlo:dhi].reshape(DC, 1)),
                "bkv": np.ascontiguousarray(bk[dlo:dhi].reshape(DC, 1)),
                "bvv": np.ascontiguousarray(bv[dlo:dhi].reshape(DC, 1)),
                "lamn": np.ascontiguousarray(
                    np.broadcast_to((-lam[h0 : h0 + HPC])[None, :], (128, HPC))
                ),
                "vones": vones_arr,
                "identr": ident_arr,
                "gnc": gnc_arr,
                "gwv": np.ascontiguousarray(gw[h0 : h0 + HPC].reshape(DC, 1)),
                "gbv": np.ascontiguousarray(gb[h0 : h0 + HPC].reshape(DC, 1)),
            }
        )

    def finish(results):
        o1 = np.zeros((NT, DIM), np.float64)
        o2 = np.zeros((NT, DIM), np.float64)
        for r in results:
            o1 += r["out1p"].astype(np.float64)
            o2 += r["out2p"].astype(np.float64)
        o1 = (o1 + bo).astype(f32).reshape(B, N, DIM)
        o2 = (o2 + bo).astype(f32).reshape(B, N, DIM)
        return o1, o2

    return in_maps, finish


def kernel(x1, x2, Wq, bq, Wk, bk, Wv, bv, Wo, bo,
           lq1, lk1, lq2, lk2, gn_w, gn_b):
    global LAST_EXEC_NS
    in_maps, finish = _host_prep(
        x1, x2, Wq, bq, Wk, bk, Wv, bv, Wo, bo,
        lq1, lk1, lq2, lk2, gn_w, gn_b,
    )
    nc = _get_program()
    trace = os.environ.get("BASS_KERNEL_TRACE", "0") == "1"
    res = run_bass_kernel_spmd(
        nc, in_maps, core_ids=list(range(NCORES)), trace=trace
    )
    LAST_EXEC_NS = res.exec_time_ns
    return finish(res.results)


# revision 32
# speedup vs baseline: 1.0108x; 1.0005x over previous
"""Differential cross-attention Bass kernel for 8 Trainium2 NeuronCores.

Sharding: heads are split across cores (2 of 16 heads per core). Each core
computes Q/K/V projections for its head slice, both N x N differential score
maps for its (batch, head) units, softmax (no max-subtraction; scores are
O(1) so exp is safe), attn = a1 - lam*a2, GroupNorm per (b, h), and a partial
output projection against its 128-column slice of Wo. The host sums the 8
partial outputs and adds the output bias.

Performance-critical design notes (v2):
- All big matmuls are fp16 with FULL k=128 contraction (HAM clock governor
  ignores <128-row matmuls). K stays dim-major as lhsT; Q is stored as four
  zero-masked variants so every score matmul is a standard 128x128 matmul.
- The exp stream (67M PSUM f32 elements/core) is THE co-bottleneck with the
  PE: it is split across BOTH ScalarE (exact spline exp, map1 columns plus a
  tunable slice of map2) and the DVE (map2 columns via a one-instruction
  Schraudolph: i16 = round(s*A + B), bits reinterpreted as fp16; the e^K
  range-shift and the sawtooth's mean cancel in the softmax division, and
  the remaining +-2.9% sawtooth error only enters scaled by lambda~0.36).
- All staging after PSUM is fp16: o12 is evacuated by ScalarE as fp16, the
  per-block transposes and tail transposes run at 1 cyc/row instead of 2,
  and the combine/affine DVE ops hit the 2x_1P packed mode. Partial outputs
  DMA to DRAM as fp16 (halves the output traffic; host sums in f64).
- Per-block combine: ONE batched reciprocal over all 8 (sub, map)
  denominators (strided AP into the single fp16 transpose tile), then
  ts+stt per sub. GroupNorm stats stay off PE/ACT (GpSimd all-reduce, DVE
  bit-trick rsqrt), computed per-head right after the head's blocks.
- Unit tails are split in half (h0 transposes+affine | h1 + Wo + output) and
  popped at two points of the NEXT unit; the LAST unit pops its h0 half
  during its own h1 blocks, shrinking the end-of-kernel half-clock window.
- Projection phase: x/wq DMAs are issued first, the q-variant zero-fills run
  on the idle GpSimd engine, and PSUM evacuation is split between ScalarE
  (Identity with per-partition bias AP) and the DVE so neither engine gates
  the x-chunk pipeline.
"""

import os
import sys
from contextlib import ExitStack

import numpy as np

for _p in ("/opt/trn_rl_repo", "/opt/pypackages"):
    if os.path.isdir(_p) and _p not in sys.path:
        sys.path.append(_p)

import concourse.bass as bass
import concourse.bass_isa as bass_isa
import concourse.tile as tile
from concourse import bacc, mybir
from concourse.bass_utils import run_bass_kernel_spmd

# ---- problem constants (hardcoded per contest contract) ----
B, N, DIM, H, HEAD, HALF = 2, 2048, 1024, 16, 64, 32
SCALE = HALF ** -0.5
LAMBDA_INIT = 0.8 - 0.6 * float(np.exp(-0.3 * (2 - 1)))
EPS = 1e-5
NCORES = 8
HPC = H // NCORES          # 2 heads per core
DC = HPC * HEAD            # 128 feature dims per core
NT = B * N                 # 4096 tokens
F32 = mybir.dt.float32
F16 = mybir.dt.float16
I16 = mybir.dt.int16

_DTMAP = {
    "float16": mybir.dt.float16,
    "bfloat16": mybir.dt.bfloat16,
    "float32r": mybir.dt.float32r,
    "float32": mybir.dt.float32,
}
MM_DT = _DTMAP[os.environ.get("BASS_MM_DT", "float16")]

# exp split: ACT handles map1 (cols 0:512 of each score tile) exactly; the
# DVE handles map2 (cols 512:1024) via Schraudolph. Separate p1/p2 tiles so
# the two engines run concurrently (a shared tile serializes the writers).
# Schraudolph fp16 constants: bits = v*SA + SB, v = s*SCALE (folded into SA)
SHIFT_K = 1.0              # extra e^K factor, cancels per-column in softmax
# (K=1: max fp16-staged denominator ~22k on the actual data, 3x margin;
#  negative scores stay out of fp16-subnormal territory down to v=-9.9)
SCHR_C = float(os.environ.get("BASS_SCHR_C", "-0.0434"))
SA = float(1024.0 / np.log(2.0)) * SCALE
SB = float((15.0 + SCHR_C) * 1024.0 + SHIFT_K * 1024.0 / np.log(2.0))

LAST_EXEC_NS = None
_PROG_CACHE = {}


def _build_kernel(nc):
    AF = mybir.ActivationFunctionType
    ALU = mybir.AluOpType
    AX = mybir.AxisListType

    x1T = nc.dram_tensor("x1T", (DIM, NT), MM_DT, kind="ExternalInput").ap()
    x2T = nc.dram_tensor("x2T", (DIM, NT), MM_DT, kind="ExternalInput").ap()
    wqT = nc.dram_tensor("wqT", (DIM, DC), MM_DT, kind="ExternalInput").ap()
    wkT = nc.dram_tensor("wkT", (DIM, DC), MM_DT, kind="ExternalInput").ap()
    wvT = nc.dram_tensor("wvT", (DIM, DC), MM_DT, kind="ExternalInput").ap()
    woT = nc.dram_tensor("woT", (DC, DIM), MM_DT, kind="ExternalInput").ap()
    bqv = nc.dram_tensor("bqv", (DC, 1), F32, kind="ExternalInput").ap()
    bkv = nc.dram_tensor("bkv", (DC, 1), F32, kind="ExternalInput").ap()
    bvv = nc.dram_tensor("bvv", (DC, 1), F32, kind="ExternalInput").ap()
    lamn = nc.dram_tensor("lamn", (128, HPC), F32, kind="ExternalInput").ap()
    vones = nc.dram_tensor("vones", (128, 32, 65), MM_DT, kind="ExternalInput").ap()
    identr = nc.dram_tensor("identr", (128, 128), MM_DT, kind="ExternalInput").ap()
    gwv = nc.dram_tensor("gwv", (DC, 1), F32, kind="ExternalInput").ap()
    gbv = nc.dram_tensor("gbv", (DC, 1), F32, kind="ExternalInput").ap()
    gnc = nc.dram_tensor(
        "gnc", (128, 2), mybir.dt.uint32, kind="ExternalInput"
    ).ap()
    out1p = nc.dram_tensor("out1p", (NT, DIM), F16, kind="ExternalOutput").ap()
    out2p = nc.dram_tensor("out2p", (NT, DIM), F16, kind="ExternalOutput").ap()

    with tile.TileContext(nc) as tc, ExitStack() as top:
        consts = top.enter_context(tc.tile_pool(name="consts", bufs=1))
        qkpool = top.enter_context(tc.tile_pool(name="qkpool", bufs=1))
        vpool = top.enter_context(tc.tile_pool(name="vpool", bufs=1))

        # ---- constants; wq + the first x chunk DMA first so the first
        # projection matmul starts as early as possible
        wq_t = consts.tile([128, 8, DC], MM_DT, tag="wq")
        wk_t = consts.tile([128, 8, DC], MM_DT, tag="wk")
        wv_t = consts.tile([128, 8, DC], MM_DT, tag="wv")
        nc.sync.dma_start(wq_t, wqT.rearrange("(kc p) d -> p kc d", p=128))
        x1Tr = x1T.rearrange("(kc p) t -> p kc t", p=128)
        x2Tr = x2T.rearrange("(kc p) t -> p kc t", p=128)
        xpre = consts.tile([128, 8, 1024], MM_DT, tag="xpre")
        nc.sync.dma_start(xpre, x1Tr[:, :, 0:1024])
        nc.sync.dma_start(wk_t, wkT.rearrange("(kc p) d -> p kc d", p=128))
        nc.sync.dma_start(wv_t, wvT.rearrange("(kc p) d -> p kc d", p=128))
        bq_t = consts.tile([DC, 1], F32, tag="bq")
        bk_t = consts.tile([DC, 1], F32, tag="bk")
        bv_t = consts.tile([DC, 1], F32, tag="bv")
        nc.sync.dma_start(bq_t, bqv)
        nc.sync.dma_start(bk_t, bkv)
        nc.sync.dma_start(bv_t, bvv)
        identr_t = consts.tile([128, 128], MM_DT, tag="identr")
        nc.sync.dma_start(identr_t, identr)
        wo_t = consts.tile([DC, DIM], MM_DT, tag="wo")
        nc.sync.dma_start(wo_t, woT)
        lam_t = consts.tile([128, HPC], F32, tag="lam")
        nc.sync.dma_start(lam_t, lamn)
        gw_t = consts.tile([DC, 1], F32, tag="gw")
        gb_t = consts.tile([DC, 1], F32, tag="gb")
        nc.sync.dma_start(gw_t, gwv)
        nc.sync.dma_start(gb_t, gbv)
        # uint32 constants for the DVE-only rsqrt: [0x5F3759DF magic, 1]
        gnc_t = consts.tile([128, 2], mybir.dt.uint32, tag="gnc")
        nc.sync.dma_start(gnc_t, gnc)

        # K dim-major; Q as 4 zero-masked variants per tensor (head x half)
        k1_t = qkpool.tile([128, NT], MM_DT, tag="k1")
        k2_t = qkpool.tile([128, NT], MM_DT, tag="k2")
        q1_v = [qkpool.tile([128, 2, NT], MM_DT, name=f"q1v{i}", tag=f"q1v{i}")
                for i in range(HPC)]
        q2_v = [qkpool.tile([128, 2, NT], MM_DT, name=f"q2v{i}", tag=f"q2v{i}")
                for i in range(HPC)]
        # variant zero-fill on the otherwise-idle GpSimd engine (keeps the
        # 4x ~7us memsets off the DVE's critical path during warmup)
        for v in q1_v + q2_v:
            nc.gpsimd.memset(v, 0.0)
        # V token-major: (tok 128, chunk 32, [64 h0 | 1 | 64 h1 | 1 | 63 pad])
        v1_t = vpool.tile([128, 32, 193], MM_DT, tag="v1")
        v2_t = vpool.tile([128, 32, 193], MM_DT, tag="v2")
        # ================= phase P: projections =================
        with ExitStack() as ph:
            xin = ph.enter_context(tc.tile_pool(name="xin", bufs=3))
            pqk = ph.enter_context(tc.tile_pool(name="pqk", bufs=3, space="PSUM"))
            pv = ph.enter_context(tc.tile_pool(name="pv", bufs=2, space="PSUM"))
            for xTr, qv, kd, vd in (
                (x1Tr, q1_v, k1_t, v1_t), (x2Tr, q2_v, k2_t, v2_t)
            ):
                for tcn in range(4):
                    ts0 = tcn * 1024
                    if xTr is x1Tr and tcn == 0:
                        xt = xpre  # prefetched before the consts DMAs
                    else:
                        xt = xin.tile([128, 8, 1024], MM_DT, tag="x")
                        nc.sync.dma_start(xt, xTr[:, :, ts0 : ts0 + 1024])
                    vstage = xin.tile([128, 1024], MM_DT, tag="vs")
                    for wt, bt, dst in (
                        (wq_t, bq_t, None), (wk_t, bk_t, kd), (wv_t, bv_t, vstage)
                    ):
                        ps = pqk.tile([128, 1024], F32, tag="qk")
                        for kc in range(8):
                            for jh in range(2):
                                nc.tensor.matmul(
                                    ps[:, jh * 512 : (jh + 1) * 512],
                                    lhsT=wt[:, kc, :],
                                    rhs=xt[:, kc, jh * 512 : (jh + 1) * 512],
                                    start=(kc == 0),
                                    stop=(kc == 7),
                                )
                        if dst is None:
                            # Q: scatter rows into the zero-masked head pairs
                            # (split ACT/DVE so neither engine gates the loop)
                            for h in range(HPC):
                                for j in range(2):
                                    hs = slice(h * 64 + j * 32, h * 64 + j * 32 + 32)
                                    dstap = qv[h][hs, j, ts0 : ts0 + 1024]
                                    if j == 0:
                                        nc.scalar.activation(
                                            dstap, ps[hs, :], AF.Identity,
                                            bias=bt[hs, :],
                                        )
                                    else:
                                        nc.vector.tensor_scalar(
                                            dstap, ps[hs, :], bt[hs, :], None,
                                            ALU.add,
                                        )
                        else:
                            out_ap = (
                                dst if dst is vstage else dst[:, ts0 : ts0 + 1024]
                            )
                            nc.scalar.activation(
                                out_ap, ps, AF.Identity, bias=bt
                            )
                    # transpose V chunk to token-major and scatter into V tile
                    for sc in range(8):
                        tp = pv.tile([128, 128], MM_DT, tag="v")
                        nc.tensor.transpose(
                            tp, vstage[:, sc * 128 : (sc + 1) * 128], identr_t
                        )
                        sg = tcn * 8 + sc
                        nc.vector.tensor_copy(vd[:, sg, 0:64], tp[:, 0:64])
                        nc.vector.tensor_copy(vd[:, sg, 65:129], tp[:, 64:128])
            for vd in (v1_t, v2_t):
                nc.sync.dma_start(vd[:, :, 64:65], vones[:, :, 0:1])
                nc.sync.dma_start(vd[:, :, 129:130], vones[:, :, 1:2])
                nc.sync.dma_start(vd[:, :, 130:193], vones[:, :, 2:65])
        # ================= phase A: attention =================
        with ExitStack() as ph:
            pp = ph.enter_context(tc.tile_pool(name="pp", bufs=6))
            osb = ph.enter_context(tc.tile_pool(name="osb", bufs=2))
            ocomb = ph.enter_context(tc.tile_pool(name="ocomb", bufs=2))
            otp = ph.enter_context(tc.tile_pool(name="otp", bufs=2))
            wos = ph.enter_context(tc.tile_pool(name="wos", bufs=4))
            small = ph.enter_context(tc.tile_pool(name="small", bufs=8))
            ps_s = ph.enter_context(tc.tile_pool(name="ps_s", bufs=2, space="PSUM"))
            ps_o = ph.enter_context(tc.tile_pool(name="ps_o", bufs=1, space="PSUM"))
            ps_t = ph.enter_context(tc.tile_pool(name="ps_t", bufs=2, space="PSUM"))

            # pending tails: (T1, T2, T3) closures per unit. T1 handles subs
            # 0..7 (transpose+affine), T2 subs 8..15, T3 the Wo projection +
            # output DMA. Non-last units pop T1 at (h0,qc1) and T2+T3 at
            # (h0,qc3) of the NEXT unit. The LAST unit runs its own T1 at
            # (h1,qc2) with the h0 affine rows only (h1 stats aren't ready),
            # and finishes the rest at the very end. All transposes are full
            # 128-partition ops (64-row ops downclock the PE via HAM).
            pending = []

            def make_tails(ot, ocu, sdv, tdv, b, outp):
                def subs_pass(s0, s1, rows):
                    for sub in range(s0, s1):
                        tb = ps_t.tile([128, 128], MM_DT, tag="t")
                        nc.tensor.transpose(tb, ocu[:, sub, :], identr_t)
                        nc.vector.tensor_scalar(
                            ot[rows, sub * 128 : (sub + 1) * 128],
                            tb[rows, :], sdv[rows, :], tdv[rows, :],
                            ALU.mult, ALU.add,
                        )

                def T1():
                    subs_pass(0, 8, slice(0, 128))

                def T2():
                    subs_pass(8, 16, slice(0, 128))

                def T_endgame():
                    # last unit only: all 16 transposes run right after the
                    # final combine (they don't need GN stats), dummy
                    # matmuls keep the PE/HAM hot while the GN chain drains,
                    # then the affines release and Wo runs at full clock.
                    tbig = [ps_t.tile([128, 8, 128], MM_DT,
                                      name=f"tbig{i}", tag="t")
                            for i in range(2)]
                    for sub in range(16):
                        nc.tensor.transpose(
                            tbig[sub // 8][:, sub % 8, :], ocu[:, sub, :],
                            identr_t,
                        )
                    for dk in range(16):
                        sd = ps_s.tile([128, 1024], F32, tag="s")
                        nc.tensor.matmul(
                            sd[:, 0:512],
                            lhsT=k1_t[:, 0:128],
                            rhs=k2_t[:, 0:512],
                            start=True,
                            stop=True,
                        )
                    for sub in range(16):
                        # split affines across both engines (ACT Identity
                        # supports per-partition scale/bias APs)
                        dst = ot[:, sub * 128 : (sub + 1) * 128]
                        src = tbig[sub // 8][:, sub % 8, :]
                        if sub % 2 == 0:
                            nc.scalar.activation(
                                dst, src, AF.Identity, bias=tdv, scale=sdv
                            )
                        else:
                            nc.vector.tensor_scalar(
                                dst, src, sdv, tdv, ALU.mult, ALU.add
                            )

                def T3_piece(p):
                    def run():
                        for tl in range(4 * p, 4 * p + 4):
                            for nc2 in range(2):
                                wp = ps_t.tile([128, 512], F32, tag="t")
                                nc.tensor.matmul(
                                    wp,
                                    lhsT=ot[:, tl * 128 : (tl + 1) * 128],
                                    rhs=wo_t[:, nc2 * 512 : (nc2 + 1) * 512],
                                    start=True,
                                    stop=True,
                                )
                                ws = wos.tile([128, 512], F16, tag="ws")
                                # evacuate each wp in halves on BOTH engines
                                # concurrently: frees the 2-buf wp ring
                                # faster, which paces the whole Wo chain
                                nc.scalar.activation(
                                    ws[:, 0:256], wp[:, 0:256], AF.Copy
                                )
                                nc.vector.tensor_copy(
                                    ws[:, 256:512], wp[:, 256:512]
                                )
                                nc.sync.dma_start(
                                    outp[
                                        b * N + tl * 128 : b * N + (tl + 1) * 128,
                                        nc2 * 512 : (nc2 + 1) * 512,
                                    ],
                                    ws,
                                )
                    return run
                return (T1, T_endgame, T2,
                        [T3_piece(p) for p in range(4)])

            units = [(b, a) for b in range(B) for a in range(2)]
            STAG = 4
            NB = len(units) * HPC * 4
            blocks = []
            for ui, (b, a) in enumerate(units):
                for h in range(HPC):
                    for qc in range(4):
                        blocks.append((ui, b, a, h, qc))

            def new_unit(ui, b, a):
                qv = q1_v if a == 0 else q2_v
                kt = k2_t if a == 0 else k1_t
                vt = v2_t if a == 0 else v1_t
                outp = out1p if a == 0 else out2p
                ot = otp.tile([128, N], MM_DT, tag="ot")
                # combined, normalized attn output for BOTH heads:
                # (q 128, sub 16, [h0 64 | h1 64]) in fp16
                ocu = ocomb.tile([128, 16, 128], F16, tag="oc")
                sdv = small.tile([128, 1], F32, tag="sdv")
                tdv = small.tile([128, 1], F32, tag="tdv")
                return {
                    "ui": ui, "b": b, "a": a, "qv": qv, "kt": kt, "vt": vt,
                    "ot": ot, "ocu": ocu, "sdv": sdv, "tdv": tdv,
                    "tails": make_tails(ot, ocu, sdv, tdv, b, outp),
                }

            def emit_prologue(U, h, qc):
                # allocate this block's accumulators and emit the first STAG
                # score/exp rounds; p1 (map1, exact ACT exp) and p2 (map2,
                # DVE Schraudolph) are separate tiles so the two exp engines
                # run concurrently.
                b, kt, qv = U["b"], U["kt"], U["qv"]
                q0 = b * N + qc * 512
                o12 = ps_o.tile([128, 1024], F32, tag="o")
                bs = {
                    "U": U, "h": h, "qc": qc, "q0": q0, "o12": o12,
                    "p1": [None] * 16, "p2": [None] * 16,
                }
                for kc in range(STAG):
                    emit_scores(bs, kc)
                return bs

            def emit_scores(bs, kc):
                U, h, q0 = bs["U"], bs["h"], bs["q0"]
                kt, qv, b = U["kt"], U["qv"], U["b"]
                k0 = b * N + kc * 128
                s = ps_s.tile([128, 1024], F32, tag="s")
                # both halves against the same K slice; each matmul stays
                # within one PSUM bank.
                for j in range(2):
                    nc.tensor.matmul(
                        s[:, j * 512 : (j + 1) * 512],
                        lhsT=kt[:, k0 : k0 + 128],
                        rhs=qv[h][:, j, q0 : q0 + 512],
                        start=True,
                        stop=True,
                    )
                p1 = pp.tile([128, 512], MM_DT, tag="p1")
                p2 = pp.tile([128, 512], MM_DT, tag="p2")
                nc.scalar.activation(p1, s[:, 0:512], AF.Exp, scale=SCALE)
                nc.vector.tensor_scalar(
                    p2.bitcast(I16), s[:, 512:1024], SA, SB,
                    ALU.mult, ALU.add,
                )
                bs["p1"][kc] = p1
                bs["p2"][kc] = p2

            def emit_av(bs, kc):
                # 65-wide lhsT (V dims + ones row): only rows 0:65 of o12
                # are read downstream, and narrower stationaries halve the
                # weight-load time (HAM keys on contraction rows, 128 here).
                U, h = bs["U"], bs["h"]
                vs = U["b"] * 16 + kc
                va0 = h * 65
                for j, pt in ((0, bs["p1"][kc]), (1, bs["p2"][kc])):
                    nc.tensor.matmul(
                        bs["o12"][0:65, j * 512 : (j + 1) * 512],
                        lhsT=U["vt"][:, vs, va0 : va0 + 65],
                        rhs=pt,
                        start=(kc == 0),
                        stop=(kc == 15),
                        skip_group_check=True,
                    )

            def emit_main(bs):
                # deferred tail pieces of the PREVIOUS unit interleave here:
                # T1/T2 at (h0,qc1)/(h0,qc3), Wo in 4 pieces across the h1
                # blocks (a monolithic Wo burst paces the PE at PSUM-
                # evacuation speed and lets HAM downclock).
                h, qc = bs["h"], bs["qc"]
                for kc in range(STAG, 16):
                    emit_scores(bs, kc)
                    emit_av(bs, kc - STAG)
                for kc in range(16 - STAG, 16):
                    emit_av(bs, kc)
                if pending:
                    pt = pending[0]
                    if h == 0 and qc == 1:
                        pt[0]()              # prev unit T1
                    if h == 0 and qc == 3:
                        pt[2]()              # prev unit T2
                    if h == 1:
                        pt[3][qc]()          # prev unit T3 piece
                        if qc == 3:
                            pending.pop(0)

            def emit_evac(bs):
                # evacuate o12 PSUM -> fp16 staging on ACT (1 op), emitted
                # BEFORE the next block's exps so it doesn't queue behind
                # them on the ACT engine
                os_t = osb.tile([65, 1024], F16, tag="os")
                nc.scalar.activation(os_t, bs["o12"][0:65, :], AF.Copy)
                bs["os_t"] = os_t

            def emit_blocktail(bs):
                # transpose to (q, [sub, 65]) -- single fp16 PSUM bank,
                # 68-col stride keeps 8B alignment -- then the batched
                # combine. Emitted AFTER the next block's score prologue so
                # the PE never idles waiting for the ACT evacuation.
                U, h, qc = bs["U"], bs["h"], bs["qc"]
                hb = h * 64
                ocu = U["ocu"]
                t12 = ps_t.tile([128, 8, 68], F16, tag="t")
                for i in range(8):
                    nc.tensor.transpose(
                        t12[:, i, 0:65],
                        bs["os_t"][:, i * 128 : (i + 1) * 128],
                        identr_t[0:65, 0:65],
                    )
                # batched combine: one reciprocal for all 8 denoms. The
                # fp16->f32 copy is tiny; feeding fp16 into reciprocal
                # directly makes walrus use a low-precision fp16 divide.
                den = small.tile([128, 8], F32, tag="den")
                nc.vector.tensor_copy(den, t12[:, :, 64:65])
                rec = small.tile([128, 8], F32, tag="rec")
                nc.vector.reciprocal(rec, den)
                s2p = small.tile([128, 4], F32, tag="s2p")
                nc.vector.tensor_scalar(
                    s2p, rec[:, 4:8], lam_t[:, h : h + 1], None, ALU.mult
                )
                for i in range(4):
                    sidx = qc * 4 + i
                    tmp = small.tile([128, 64], F16, tag="tmp")
                    nc.vector.tensor_scalar(
                        tmp, t12[:, i, 0:64], rec[:, i : i + 1], None,
                        ALU.mult,
                    )
                    # ocu = (O2 * s2p) + tmp   (s2p = -lam / sum2)
                    nc.vector.scalar_tensor_tensor(
                        ocu[:, sidx, hb : hb + 64],
                        t12[:, 4 + i, 0:64],
                        s2p[:, i : i + 1],
                        tmp,
                        ALU.mult,
                        ALU.add,
                    )

            def emit_gn_stats(U, h):
                # GroupNorm stats for head h: free-dim sums ride along ACT
                # activations via accum_out (a DVE reduce on fp16 input
                # inserts big CAST instructions), then a GpSimd partition
                # all-reduce + DVE bit-trick rsqrt (seed + 2 Newton steps).
                ocu, sdv, tdv = U["ocu"], U["sdv"], U["tdv"]
                hb = h * 64
                st = small.tile([128, 2], F32, tag="st")
                sq = ocomb.tile([128, 16, 64], F16, tag="sq")
                nc.scalar.activation(
                    sq, ocu[:, :, hb : hb + 64], AF.Identity,
                    accum_out=st[:, 0:1],
                )
                nc.scalar.activation(
                    sq, ocu[:, :, hb : hb + 64], AF.Square,
                    accum_out=st[:, 1:2],
                )
                red = small.tile([128, 2], F32, tag="red")
                nc.gpsimd.partition_all_reduce(
                    red, st, 128, bass_isa.ReduceOp.add
                )
                mr = small.tile([128, 2], F32, tag="mr")
                nc.vector.tensor_scalar(
                    mr, red, 1.0 / (N * HEAD), None, ALU.mult
                )
                m2 = small.tile([128, 1], F32, tag="m2")
                nc.vector.tensor_tensor(m2, mr[:, 0:1], mr[:, 0:1], ALU.mult)
                var = small.tile([128, 1], F32, tag="var")
                nc.vector.tensor_sub(var, mr[:, 1:2], m2)
                hs = slice(h * 64, h * 64 + 64)
                veps = small.tile([128, 1], F32, tag="veps")
                nc.vector.tensor_scalar(veps, var, EPS, None, ALU.add)
                vsh = small.tile([128, 1], mybir.dt.uint32, tag="vsh")
                nc.vector.tensor_scalar(
                    vsh, veps.bitcast(mybir.dt.uint32),
                    gnc_t[:, 1:2], None, ALU.logical_shift_right,
                )
                y0i = small.tile([128, 1], mybir.dt.uint32, tag="y0i")
                nc.vector.tensor_tensor(y0i, gnc_t[:, 0:1], vsh, ALU.subtract)
                vh = small.tile([128, 1], F32, tag="vh")
                nc.vector.tensor_scalar(vh, veps, 0.5, None, ALU.mult)
                y = y0i.bitcast(F32)
                for _ in range(2):  # Newton: y*(1.5-vh*y^2)
                    t1n = small.tile([128, 1], F32, tag="nt1")
                    nc.vector.tensor_tensor(t1n, y, y, ALU.mult)
                    t2n = small.tile([128, 1], F32, tag="nt2")
                    nc.vector.tensor_tensor(t2n, vh, t1n, ALU.mult)
                    t3n = small.tile([128, 1], F32, tag="nt3")
                    nc.vector.tensor_scalar(
                        t3n, t2n, 1.5, -1.0, ALU.subtract, ALU.mult
                    )
                    yn = small.tile([128, 1], F32, tag="yn")
                    nc.vector.tensor_tensor(yn, y, t3n, ALU.mult)
                    y = yn
                tmp1 = small.tile([128, 1], F32, tag="tmp1")
                nc.vector.tensor_tensor(
                    sdv[hs, :], gw_t[hs, :], y[hs, :], ALU.mult
                )
                nc.vector.tensor_tensor(
                    tmp1[hs, :], mr[hs, 0:1], sdv[hs, :], ALU.mult
                )
                nc.vector.tensor_sub(tdv[hs, :], gb_t[hs, :], tmp1[hs, :])

            # ---- the block pipeline: per iteration, evacuate the previous
            # block, start the next block's scores, then finish the previous
            # block's transposes/combine so the PE never waits on the ACT
            # evacuation at block boundaries.
            U = None
            prev = None
            for bi in range(NB + 1):
                blk = blocks[bi] if bi < NB else None
                if prev is not None:
                    emit_evac(prev)
                bs = None
                if blk is not None:
                    ui, b, a, h, qc = blk
                    if h == 0 and qc == 0:
                        U = new_unit(ui, b, a)
                    bs = emit_prologue(U, h, qc)
                if prev is not None:
                    emit_blocktail(prev)
                    pU, ph, pqc = prev["U"], prev["h"], prev["qc"]
                    if pqc == 3:
                        emit_gn_stats(pU, ph)
                        if ph == 1 and pU["ui"] < len(units) - 1:
                            pending.append(pU["tails"])
                if bs is not None:
                    emit_main(bs)
                    if bi == NB - 1:
                        # bridge the final combine gap so HAM stays at full
                        # clock into the endgame
                        for dk in range(8):
                            sd = ps_s.tile([128, 1024], F32, tag="s")
                            nc.tensor.matmul(
                                sd[:, 0:512],
                                lhsT=U["kt"][:, 0:128],
                                rhs=U["qv"][0][:, 0, 0:512],
                                start=True,
                                stop=True,
                            )
                    prev = bs
            # final unit: transposes + dummies + affines, then Wo
            my_tails = U["tails"]
            my_tails[1]()
            for piece in my_tails[3]:
                piece()
    return nc


def _get_program():
    key = ("prog", str(MM_DT), SCHR_C)
    if key not in _PROG_CACHE:
        nc = bacc.Bacc("TRN2", target_bir_lowering=False, debug=False)
        _build_kernel(nc)
        nc.compile()
        _PROG_CACHE[key] = nc
    return _PROG_CACHE[key]


def _host_prep(x1, x2, Wq, bq, Wk, bk, Wv, bv, Wo, bo,
               lq1, lk1, lq2, lk2, gn_w, gn_b):
    f32 = np.float32
    x1 = np.asarray(x1, f32)
    x2 = np.asarray(x2, f32)
    lam = (
        np.exp((np.asarray(lq1, f32) * np.asarray(lk1, f32)).sum(-1))
        - np.exp((np.asarray(lq2, f32) * np.asarray(lk2, f32)).sum(-1))
        + f32(LAMBDA_INIT)
    ).astype(f32)  # (H,)
    sc = f32(1.0 - LAMBDA_INIT)
    gw = (np.asarray(gn_w, f32) * sc).reshape(H, HEAD)
    gb = (np.asarray(gn_b, f32) * sc).reshape(H, HEAD)
    Wq, Wk, Wv, Wo = (np.asarray(w, f32) for w in (Wq, Wk, Wv, Wo))
    bq, bk, bv, bo = (np.asarray(v_, f32) for v_ in (bq, bk, bv, bo))

    mdt = mybir.dt.np(MM_DT)
    x1T = np.ascontiguousarray(x1.reshape(NT, DIM).T).astype(mdt)
    x2T = np.ascontiguousarray(x2.reshape(NT, DIM).T).astype(mdt)
    vones_arr = np.zeros((128, 32, 65), mdt)
    vones_arr[:, :, 0:2] = 1.0
    ident_arr = np.eye(128, dtype=mdt)
    gnc_arr = np.ascontiguousarray(
        np.broadcast_to(
            np.array([[0x5F3759DF, 1]], np.uint32), (128, 2)
        )
    )

    in_maps = []
    for c in range(NCORES):
        dlo, dhi = c * DC, (c + 1) * DC
        h0 = c * HPC
        in_maps.append(
            {
                "x1T": x1T,
                "x2T": x2T,
                "wqT": np.ascontiguousarray(Wq[dlo:dhi, :].T).astype(mdt),
                "wkT": np.ascontiguousarray(Wk[dlo:dhi, :].T).astype(mdt),
                "wvT": np.ascontiguousarray(Wv[dlo:dhi, :].T).astype(mdt),
                "woT": np.ascontiguousarray(Wo[:, dlo:dhi].T).astype(mdt),
                "bqv": np.ascontiguousarray(bq[dlo:dhi].reshape(DC, 1)),
                "bkv": np.ascontiguousarray(bk[dlo:dhi].reshape(DC, 1)),
                "bvv": np.ascontiguousarray(bv[dlo:dhi].reshape(DC, 1)),
                "lamn": np.ascontiguousarray(
                    np.broadcast_to((-lam[h0 : h0 + HPC])[None, :], (128, HPC))
                ),
                "vones": vones_arr,
                "identr": ident_arr,
                "gnc": gnc_arr,
                "gwv": np.ascontiguousarray(gw[h0 : h0 + HPC].reshape(DC, 1)),
                "gbv": np.ascontiguousarray(gb[h0 : h0 + HPC].reshape(DC, 1)),
            }
        )

    def finish(results):
        o1 = np.zeros((NT, DIM), np.float64)
        o2 = np.zeros((NT, DIM), np.float64)
        for r in results:
            o1 += r["out1p"].astype(np.float64)
            o2 += r["out2p"].astype(np.float64)
        o1 = (o1 + bo).astype(f32).reshape(B, N, DIM)
        o2 = (o2 + bo).astype(f32).reshape(B, N, DIM)
        return o1, o2

    return in_maps, finish


def kernel(x1, x2, Wq, bq, Wk, bk, Wv, bv, Wo, bo,
           lq1, lk1, lq2, lk2, gn_w, gn_b):
    global LAST_EXEC_NS
    in_maps, finish = _host_prep(
        x1, x2, Wq, bq, Wk, bk, Wv, bv, Wo, bo,
        lq1, lk1, lq2, lk2, gn_w, gn_b,
    )
    nc = _get_program()
    trace = os.environ.get("BASS_KERNEL_TRACE", "0") == "1"
    res = run_bass_kernel_spmd(
        nc, in_maps, core_ids=list(range(NCORES)), trace=trace
    )
    LAST_EXEC_NS = res.exec_time_ns
    return finish(res.results)
